# revision 13
# baseline (speedup 1.0000x reference)
"""Trainium2 Bass kernel for nn_Net_34359738709 (spiking RNN).

Model (per timestep t, reference semantics):
    cur1  = x_t @ W1.T + b1                      # [B, NH] big matmul, t-independent
    mem1  = beta1c*mem1 + cur1 + spk1 @ V.T + Vb - spk1*THRESH
    spk1  = (mem1 - THRESH > 0)
    cur2  = spk1 @ W2.T + b2
    mem2  = BETA2*mem2 + cur2 - spk2_prev*THRESH
    spk2  = (mem2 - THRESH > 0)
outputs: (spk2_rec, mem2_rec), each [T, B, NOUT]

Strategy: data-parallel over batch (B=64 -> 8 cores x 8). The x @ W1.T
matmul (21 GFLOP) is hoisted out of the time scan and computed as
cur1.T[NH, T*BL] = W1 @ x.T, accumulated over 256 K-chunks of 128 in one
PSUM bank per column tile. fp32 accuracy at bf16 speed via a hi/lo split:
x = x_hi + x_lo, W1 = w_hi + w_lo (each bf16), cur1 ~= w_hi@x_hi +
w_hi@x_lo + w_lo@x_hi (verified exact spike pattern vs fp32). Two column
tiles (34/16 timesteps): the first tile's sequential scan overlaps the
second tile's matmuls. Per scan step, layer 1 runs one augmented
128-contraction matmul (lhsT rows 0..99 = (V-I).T, row 100 = Vb, rhs =
[spk1; 1; 0]) plus three vector ops; layer 2 uses a per-burst batched
W2 matmul then a vector-only 3-op chain per step. All inputs are
host-pre-arranged into exact SBUF layouts so DMA runs long-contiguous.
"""

import sys

if "/opt/trn_rl_repo" not in sys.path:
    sys.path.insert(0, "/opt/trn_rl_repo")

import numpy as np

# Problem shapes (hardcoded per contract)
T, B, NIN, NH, NOUT = 50, 64, 32768, 100, 11
NCORES = 8
BL = B // NCORES          # 8 batch rows per core
TBL = T * BL              # 400 columns (t-major: col = t*BL + b)
KP = 128                  # contraction partition size
KCH = NIN // KP           # 256 K-chunks
COL_TILES = [256, 144]    # ncols per column tile, each % BL == 0
X_GROUPS = [2, 2, 4] + [8] * 31   # K-chunks per x dma_start (sums to 256)
THRESH = 1.0
BETA2 = 0.9753

PRECISION = "bf16x2"      # "fp32" | "bf16x2"

_PROG = {}


def _build_body(tc, nc, mybir, aps, precision):
    f32 = mybir.dt.float32
    Alu = mybir.AluOpType
    mm_dt = {"fp32": f32, "bf16x2": mybir.dt.bfloat16}[precision]
    NS = 2 if precision == "bf16x2" else 1
    xts, w1f, a1, w2a, b1, bet, s1init, spk_o, mem_o = aps

    from contextlib import ExitStack

    stack = ExitStack()
    const_pool = stack.enter_context(tc.tile_pool(name="const", bufs=1))
    state_pool = stack.enter_context(tc.tile_pool(name="state", bufs=1))
    xpool = stack.enter_context(tc.tile_pool(name="xg", bufs=5))
    curpool = stack.enter_context(tc.tile_pool(name="cur", bufs=2))
    ps_a = stack.enter_context(tc.tile_pool(name="psa", bufs=2, space="PSUM"))
    ps_b = stack.enter_context(tc.tile_pool(name="psb", bufs=2, space="PSUM"))
    ps_s1 = stack.enter_context(tc.tile_pool(name="pss1", bufs=2, space="PSUM"))
    ps_c2 = stack.enter_context(tc.tile_pool(name="psc2", bufs=2, space="PSUM"))

    MAXC = max(COL_TILES)
    MAXG = max(X_GROUPS)

    # ---- small constants (scalar-engine HWDGE ring, ahead of W1 groups) ----
    a1sb = const_pool.tile([KP, NH], f32)
    nc.scalar.dma_start(a1sb[:], a1)
    w2sb = const_pool.tile([KP, NOUT], f32)
    nc.scalar.dma_start(w2sb[:], w2a)
    b1sb = const_pool.tile([NH, 1], f32)
    nc.scalar.dma_start(b1sb[:], b1)
    betnsb = const_pool.tile([NH, 1], f32)   # NEGATED clipped beta1
    nc.scalar.dma_start(betnsb[:], bet)
    # spk1 ring buffer: col block t+1 = spk1 after step t; rows 100..127
    # carry the [1; 0-pad] augmentation for every column (from s1init).
    spk1buf = state_pool.tile([KP, BL + TBL], f32)
    nc.scalar.dma_start(spk1buf[:], s1init)

    # W1 resident in SBUF, exact matmul layout [128, NS*KCH*NH] (bf16 hi|lo)
    w1sb = const_pool.tile([KP, NS * KCH * NH], mm_dt)

    def w1_chunk(c, s=0):
        base = (c * NS + s) * NH
        return w1sb[:, base:base + NH]

    # ---- state ----
    mem1 = state_pool.tile([NH, BL], f32)
    nc.vector.memset(mem1[:], 0.0)
    m2rec = state_pool.tile([NOUT, BL + TBL], f32)
    s2rec = state_pool.tile([NOUT, BL + TBL], f32)
    nc.vector.memset(m2rec[:, 0:BL], 0.0)
    nc.vector.memset(s2rec[:, 0:BL], 0.0)

    tmpneg = state_pool.tile([NH, BL], f32)

    t_global = 0
    for j, cols in enumerate(COL_TILES):
        xt = xts[j]           # [128, NS*KCH*cols] dram, matmul-ready
        # psA accumulates [w_hi@x_hi | w_hi@x_lo] (N=2*cols); psB w_lo@x_hi
        psa = ps_a.tile([NH, 2 * MAXC], f32)
        psb = ps_b.tile([NH, MAXC], f32)
        c0 = 0
        for g, gch in enumerate(X_GROUPS):
            if j == 0:
                # stream the matching W1 chunk range on the scalar ring
                w0, w1n = c0 * NS * NH, (c0 + gch) * NS * NH
                nc.scalar.dma_start(w1sb[:, w0:w1n], w1f[:, w0:w1n])
            xg = xpool.tile([KP, NS * MAXG * MAXC], mm_dt)
            gsz = NS * gch * cols
            if j == 0:
                dma_eng = (nc.sync, nc.gpsimd)[g % 2]
            else:
                dma_eng = (nc.sync, nc.gpsimd, nc.scalar)[g % 3]
            dma_eng.dma_start(xg[:, :gsz], xt[:, c0 * NS * cols:(c0 + gch) * NS * cols])

            for ci in range(gch):
                c = c0 + ci
                if NS == 1:
                    nc.tensor.matmul(
                        psa[:, :cols], lhsT=w1_chunk(c),
                        rhs=xg[:, ci * cols:(ci + 1) * cols],
                        start=(c == 0), stop=(c == KCH - 1))
                else:
                    # hi/lo split: one MM covers hh|hl (concat cols), one lh
                    nc.tensor.matmul(
                        psa[:, :2 * cols], lhsT=w1_chunk(c, 0),
                        rhs=xg[:, ci * 2 * cols:(ci + 1) * 2 * cols],
                        start=(c == 0), stop=(c == KCH - 1))
                    nc.tensor.matmul(
                        psb[:, :cols], lhsT=w1_chunk(c, 1),
                        rhs=xg[:, ci * 2 * cols:ci * 2 * cols + cols],
                        start=(c == 0), stop=(c == KCH - 1))
            c0 += gch
        cur = curpool.tile([NH, MAXC], f32)
        if NS == 1:
            nc.vector.tensor_scalar_add(cur[:, :cols], psa[:, :cols], b1sb[:, 0:1])
        else:
            # cur = psa_hh + b1, += psa_hl, += psb (one PSUM read per op)
            nc.vector.tensor_scalar_add(cur[:, :cols], psa[:, :cols],
                                        b1sb[:, 0:1])
            nc.vector.tensor_add(cur[:, :cols], cur[:, :cols],
                                 psa[:, cols:2 * cols])
            nc.vector.tensor_add(cur[:, :cols], cur[:, :cols], psb[:, :cols])

        # ---- layer-1 sequential scan for this tile's timesteps ----
        # tmpneg = -beta*mem1 - cur_t  (independent of the V matmul)
        # spk1  = (rec - 1) > tmpneg   (single fused op after the matmul)
        # mem1  = rec - tmpneg
        nsteps = cols // BL
        nc.vector.scalar_tensor_tensor(
            tmpneg[:], mem1[:], betnsb[:, 0:1], cur[:, 0:BL],
            Alu.mult, Alu.subtract)
        for k in range(nsteps):
            t = t_global + k
            rec = ps_s1.tile([NH, BL], f32)
            nc.tensor.matmul(rec[:], lhsT=a1sb[:, :],
                             rhs=spk1buf[:, t * BL:(t + 1) * BL],
                             start=True, stop=True)
            nc.vector.scalar_tensor_tensor(
                spk1buf[0:NH, (t + 1) * BL:(t + 2) * BL], rec[:],
                THRESH, tmpneg[:], Alu.subtract, Alu.is_gt)
            nc.vector.tensor_sub(mem1[:], rec[:], tmpneg[:])
            if k + 1 < nsteps:
                nc.vector.scalar_tensor_tensor(
                    tmpneg[:], mem1[:], betnsb[:, 0:1],
                    cur[:, (k + 1) * BL:(k + 2) * BL], Alu.mult, Alu.subtract)

        # ---- layer 2: one batched matmul, then vector-only chain ----
        c2 = ps_c2.tile([NOUT, MAXC], f32)
        nc.tensor.matmul(c2[:, :cols], lhsT=w2sb[:, :],
                         rhs=spk1buf[:, (t_global + 1) * BL:
                                     (t_global + 1 + nsteps) * BL],
                         start=True, stop=True)
        for k in range(nsteps):
            t = t_global + k
            mprev = m2rec[:, t * BL:(t + 1) * BL]
            mcur = m2rec[:, (t + 1) * BL:(t + 2) * BL]
            sprev = s2rec[:, t * BL:(t + 1) * BL]
            scur = s2rec[:, (t + 1) * BL:(t + 2) * BL]
            nc.vector.scalar_tensor_tensor(
                mcur, mprev, BETA2, sprev, Alu.mult, Alu.subtract)
            nc.vector.tensor_add(mcur, mcur, c2[:, k * BL:(k + 1) * BL])
            nc.vector.tensor_scalar(scur, mcur, THRESH, None, Alu.is_gt)
        t_global += nsteps

    nc.sync.dma_start(spk_o[:], s2rec[:, BL:BL + TBL])
    nc.sync.dma_start(mem_o[:], m2rec[:, BL:BL + TBL])
    stack.close()


def build_program(precision=None):
    precision = precision or PRECISION
    if precision in _PROG:
        return _PROG[precision]
    import concourse.tile as tile
    from concourse import bacc, mybir

    f32 = mybir.dt.float32
    mm_dt = {"fp32": f32, "bf16x2": mybir.dt.bfloat16}[precision]
    NS = 2 if precision == "bf16x2" else 1
    nc = bacc.Bacc("TRN2", target_bir_lowering=False, debug=False,
                   num_devices=NCORES)
    xts = [nc.dram_tensor(f"xt{j}", [KP, NS * KCH * cols], mm_dt,
                          kind="ExternalInput").ap()
           for j, cols in enumerate(COL_TILES)]
    w1f = nc.dram_tensor("w1f", [KP, NS * KCH * NH], mm_dt,
                         kind="ExternalInput").ap()
    a1 = nc.dram_tensor("a1", [KP, NH], f32, kind="ExternalInput").ap()
    w2a = nc.dram_tensor("w2a", [KP, NOUT], f32, kind="ExternalInput").ap()
    b1 = nc.dram_tensor("b1", [NH, 1], f32, kind="ExternalInput").ap()
    bet = nc.dram_tensor("bet", [NH, 1], f32, kind="ExternalInput").ap()
    s1init = nc.dram_tensor("s1init", [KP, BL + TBL], f32,
                            kind="ExternalInput").ap()
    spk_o = nc.dram_tensor("spk", [NOUT, TBL], f32, kind="ExternalOutput").ap()
    mem_o = nc.dram_tensor("mem", [NOUT, TBL], f32, kind="ExternalOutput").ap()
    aps = (xts, w1f, a1, w2a, b1, bet, s1init, spk_o, mem_o)
    with tile.TileContext(nc) as tc:
        _build_body(tc, nc, mybir, aps, precision)
    nc.compile()
    _PROG[precision] = nc
    return nc


def _mm_layout(kxn, nsplit):
    """[K=NIN, N] fp32 -> [128, nsplit*KCH*N] in matmul-ready order
    (chunk-major, hi|lo interleaved per chunk)."""
    import ml_dtypes
    n = kxn.shape[1]
    v = np.ascontiguousarray(
        kxn.reshape(KCH, KP, n).transpose(1, 0, 2))     # [128, KCH, n]
    if nsplit == 1:
        return v.reshape(KP, KCH * n)
    hi = v.astype(ml_dtypes.bfloat16)
    lo = (v - hi.astype(np.float32)).astype(ml_dtypes.bfloat16)
    out = np.empty((KP, KCH, 2, n), hi.dtype)
    out[:, :, 0, :] = hi
    out[:, :, 1, :] = lo
    return np.ascontiguousarray(out).reshape(KP, 2 * KCH * n)


def prep_inputs(x, W1, b1, beta1, V, Vb, W2, b2, precision=None):
    """Host-side shard + layout prep. Returns list of per-core input dicts."""
    precision = precision or PRECISION
    nsplit = 2 if precision == "bf16x2" else 1
    f32 = np.float32
    w1f = _mm_layout(np.ascontiguousarray(W1.T, dtype=f32), nsplit)
    a1 = np.zeros((KP, NH), f32)
    a1[:NH] = (V - THRESH * np.eye(NH, dtype=f32)).T
    a1[NH] = Vb
    w2a = np.zeros((KP, NOUT), f32)
    w2a[:NH] = W2.T
    w2a[NH] = b2
    b1a = np.ascontiguousarray(b1.reshape(NH, 1), dtype=f32)
    beta = (-np.clip(beta1, 0.0, 1.0)).astype(f32).reshape(NH, 1)  # negated
    s1init = np.zeros((KP, BL + TBL), f32)
    s1init[NH] = 1.0
    # x: [T, B, NIN] -> per-core column tiles in matmul-ready layout
    xt_full = np.ascontiguousarray(x.transpose(2, 0, 1))        # [NIN, T, B]
    col_edges = np.cumsum([0] + COL_TILES)
    in_maps = []
    for c in range(NCORES):
        xTc = np.ascontiguousarray(
            xt_full[:, :, c * BL:(c + 1) * BL]).reshape(NIN, TBL)
        m = dict(w1f=w1f, a1=a1, w2a=w2a, b1=b1a, bet=beta, s1init=s1init)
        for j, cols in enumerate(COL_TILES):
            m[f"xt{j}"] = _mm_layout(
                np.ascontiguousarray(xTc[:, col_edges[j]:col_edges[j + 1]]),
                nsplit)
        in_maps.append(m)
    return in_maps


def gather_outputs(results):
    """results: list of per-core {'spk': [NOUT, TBL], 'mem': [NOUT, TBL]}."""
    spks, mems = [], []
    for r in results:
        spks.append(np.ascontiguousarray(
            r["spk"].reshape(NOUT, T, BL).transpose(1, 2, 0)))
        mems.append(np.ascontiguousarray(
            r["mem"].reshape(NOUT, T, BL).transpose(1, 2, 0)))
    spk = np.concatenate(spks, axis=1)
    mem = np.concatenate(mems, axis=1)
    return spk.astype(np.float32), mem.astype(np.float32)


def kernel(x, W1, b1, beta1, V, Vb, W2, b2, **_run_kwargs):
    from concourse import bass_utils

    precision = _run_kwargs.pop("precision", None) or PRECISION
    nc = build_program(precision)
    in_maps = prep_inputs(np.asarray(x, np.float32), np.asarray(W1, np.float32),
                          np.asarray(b1, np.float32), np.asarray(beta1, np.float32),
                          np.asarray(V, np.float32), np.asarray(Vb, np.float32),
                          np.asarray(W2, np.float32), np.asarray(b2, np.float32),
                          precision)
    res = bass_utils.run_bass_kernel_spmd(
        nc, in_maps, core_ids=list(range(NCORES)), **_run_kwargs)
    out = gather_outputs(res.results)
    kernel.last_result = res
    return out


# revision 14
# speedup vs baseline: 1.0070x; 1.0070x over previous
"""Trainium2 Bass kernel for nn_Net_34359738709 (spiking RNN).

Model (per timestep t, reference semantics):
    cur1  = x_t @ W1.T + b1                      # [B, NH] big matmul, t-independent
    mem1  = beta1c*mem1 + cur1 + spk1 @ V.T + Vb - spk1*THRESH
    spk1  = (mem1 - THRESH > 0)
    cur2  = spk1 @ W2.T + b2
    mem2  = BETA2*mem2 + cur2 - spk2_prev*THRESH
    spk2  = (mem2 - THRESH > 0)
outputs: (spk2_rec, mem2_rec), each [T, B, NOUT]

Strategy: data-parallel over batch (B=64 -> 8 cores x 8). The x @ W1.T
matmul (21 GFLOP) is hoisted out of the time scan and computed as
cur1.T[NH, T*BL] = W1 @ x.T, accumulated over 256 K-chunks of 128 in one
PSUM bank per column tile. fp32 accuracy at bf16 speed via a hi/lo split:
x = x_hi + x_lo, W1 = w_hi + w_lo (each bf16), cur1 ~= w_hi@x_hi +
w_hi@x_lo + w_lo@x_hi (verified exact spike pattern vs fp32). Two column
tiles (34/16 timesteps): the first tile's sequential scan overlaps the
second tile's matmuls. Per scan step, layer 1 runs one augmented
128-contraction matmul (lhsT rows 0..99 = (V-I).T, row 100 = Vb, rhs =
[spk1; 1; 0]) plus three vector ops; layer 2 uses a per-burst batched
W2 matmul then a vector-only 3-op chain per step. All inputs are
host-pre-arranged into exact SBUF layouts so DMA runs long-contiguous.
"""

import sys

if "/opt/trn_rl_repo" not in sys.path:
    sys.path.insert(0, "/opt/trn_rl_repo")

import numpy as np

# Problem shapes (hardcoded per contract)
T, B, NIN, NH, NOUT = 50, 64, 32768, 100, 11
NCORES = 8
BL = B // NCORES          # 8 batch rows per core
TBL = T * BL              # 400 columns (t-major: col = t*BL + b)
KP = 128                  # contraction partition size
KCH = NIN // KP           # 256 K-chunks
COL_TILES = [256, 144]    # ncols per column tile, each % BL == 0
X_GROUPS = [2, 2, 4] + [8] * 31   # K-chunks per x dma_start (sums to 256)
THRESH = 1.0
BETA2 = 0.9753

PRECISION = "bf16x2"      # "fp32" | "bf16x2"

_PROG = {}


def _build_body(tc, nc, mybir, aps, precision):
    f32 = mybir.dt.float32
    Alu = mybir.AluOpType
    mm_dt = {"fp32": f32, "bf16x2": mybir.dt.bfloat16}[precision]
    NS = 2 if precision == "bf16x2" else 1
    xts, w1f, a1, w2a, b1, bet, s1init, spk_o, mem_o = aps

    from contextlib import ExitStack

    stack = ExitStack()
    const_pool = stack.enter_context(tc.tile_pool(name="const", bufs=1))
    state_pool = stack.enter_context(tc.tile_pool(name="state", bufs=1))
    xpool = stack.enter_context(tc.tile_pool(name="xg", bufs=5))
    curpool = stack.enter_context(tc.tile_pool(name="cur", bufs=2))
    ps_a = stack.enter_context(tc.tile_pool(name="psa", bufs=2, space="PSUM"))
    ps_b = stack.enter_context(tc.tile_pool(name="psb", bufs=2, space="PSUM"))
    ps_s1 = stack.enter_context(tc.tile_pool(name="pss1", bufs=2, space="PSUM"))
    ps_c2 = stack.enter_context(tc.tile_pool(name="psc2", bufs=2, space="PSUM"))

    MAXC = max(COL_TILES)
    MAXG = max(X_GROUPS)

    # ---- small constants (scalar-engine HWDGE ring, ahead of W1 groups) ----
    a1sb = const_pool.tile([KP, NH], f32)
    nc.scalar.dma_start(a1sb[:], a1)
    w2sb = const_pool.tile([KP, NOUT], f32)
    nc.scalar.dma_start(w2sb[:], w2a)
    b1sb = const_pool.tile([NH, 1], f32)
    nc.scalar.dma_start(b1sb[:], b1)
    betnsb = const_pool.tile([NH, 1], f32)   # NEGATED clipped beta1
    nc.scalar.dma_start(betnsb[:], bet)
    # spk1 ring buffer: col block t+1 = spk1 after step t; rows 100..127
    # carry the [1; 0-pad] augmentation for every column (from s1init).
    spk1buf = state_pool.tile([KP, BL + TBL], f32)
    nc.scalar.dma_start(spk1buf[:], s1init)

    # W1 resident in SBUF, exact matmul layout [128, NS*KCH*NH] (bf16 hi|lo)
    w1sb = const_pool.tile([KP, NS * KCH * NH], mm_dt)

    def w1_chunk(c, s=0):
        base = (c * NS + s) * NH
        return w1sb[:, base:base + NH]

    # ---- state ----
    mem1 = state_pool.tile([NH, BL], f32)
    nc.vector.memset(mem1[:], 0.0)
    m2rec = state_pool.tile([NOUT, BL + TBL], f32)
    s2rec = state_pool.tile([NOUT, BL + TBL], f32)
    nc.vector.memset(m2rec[:, 0:BL], 0.0)
    nc.vector.memset(s2rec[:, 0:BL], 0.0)

    tmpneg = state_pool.tile([NH, BL], f32)

    t_global = 0
    for j, cols in enumerate(COL_TILES):
        xt = xts[j]           # [128, NS*KCH*cols] dram, matmul-ready
        # psA accumulates [w_hi@x_hi | w_hi@x_lo] (N=2*cols); psB w_lo@x_hi
        psa = ps_a.tile([NH, 2 * MAXC], f32)
        psb = ps_b.tile([NH, MAXC], f32)
        c0 = 0
        for g, gch in enumerate(X_GROUPS):
            if j == 0:
                # stream the matching W1 chunk range on the scalar ring
                w0, w1n = c0 * NS * NH, (c0 + gch) * NS * NH
                nc.scalar.dma_start(w1sb[:, w0:w1n], w1f[:, w0:w1n])
            xg = xpool.tile([KP, NS * MAXG * MAXC], mm_dt)
            gsz = NS * gch * cols
            dma_eng = (nc.sync, nc.gpsimd)[g % 2]
            dma_eng.dma_start(xg[:, :gsz], xt[:, c0 * NS * cols:(c0 + gch) * NS * cols])

            if NS == 1:
                for ci in range(gch):
                    c = c0 + ci
                    nc.tensor.matmul(
                        psa[:, :cols], lhsT=w1_chunk(c),
                        rhs=xg[:, ci * cols:(ci + 1) * cols],
                        start=(c == 0), stop=(c == KCH - 1))
            else:
                # hi/lo split: one MM covers hh|hl (concat cols), one lh.
                # All w_hi MMs of the group first, then all w_lo MMs, so the
                # PSUM write bank switches once per group, not per chunk.
                for ci in range(gch):
                    c = c0 + ci
                    nc.tensor.matmul(
                        psa[:, :2 * cols], lhsT=w1_chunk(c, 0),
                        rhs=xg[:, ci * 2 * cols:(ci + 1) * 2 * cols],
                        start=(c == 0), stop=(c == KCH - 1))
                for ci in range(gch):
                    c = c0 + ci
                    nc.tensor.matmul(
                        psb[:, :cols], lhsT=w1_chunk(c, 1),
                        rhs=xg[:, ci * 2 * cols:ci * 2 * cols + cols],
                        start=(c == 0), stop=(c == KCH - 1))
            c0 += gch
        cur = curpool.tile([NH, MAXC], f32)
        if NS == 1:
            nc.vector.tensor_scalar_add(cur[:, :cols], psa[:, :cols], b1sb[:, 0:1])
        else:
            # cur = psa_hh + b1, += psa_hl, += psb (one PSUM read per op)
            nc.vector.tensor_scalar_add(cur[:, :cols], psa[:, :cols],
                                        b1sb[:, 0:1])
            nc.vector.tensor_add(cur[:, :cols], cur[:, :cols],
                                 psa[:, cols:2 * cols])
            nc.vector.tensor_add(cur[:, :cols], cur[:, :cols], psb[:, :cols])

        # ---- layer-1 sequential scan for this tile's timesteps ----
        # tmpneg = -beta*mem1 - cur_t  (independent of the V matmul)
        # spk1  = (rec - 1) > tmpneg   (single fused op after the matmul)
        # mem1  = rec - tmpneg
        nsteps = cols // BL
        nc.vector.scalar_tensor_tensor(
            tmpneg[:], mem1[:], betnsb[:, 0:1], cur[:, 0:BL],
            Alu.mult, Alu.subtract)
        for k in range(nsteps):
            t = t_global + k
            rec = ps_s1.tile([NH, BL], f32)
            nc.tensor.matmul(rec[:], lhsT=a1sb[:, :],
                             rhs=spk1buf[:, t * BL:(t + 1) * BL],
                             start=True, stop=True)
            nc.vector.scalar_tensor_tensor(
                spk1buf[0:NH, (t + 1) * BL:(t + 2) * BL], rec[:],
                THRESH, tmpneg[:], Alu.subtract, Alu.is_gt)
            nc.vector.tensor_sub(mem1[:], rec[:], tmpneg[:])
            if k + 1 < nsteps:
                nc.vector.scalar_tensor_tensor(
                    tmpneg[:], mem1[:], betnsb[:, 0:1],
                    cur[:, (k + 1) * BL:(k + 2) * BL], Alu.mult, Alu.subtract)

        # ---- layer 2: one batched matmul, then vector-only chain ----
        c2 = ps_c2.tile([NOUT, MAXC], f32)
        nc.tensor.matmul(c2[:, :cols], lhsT=w2sb[:, :],
                         rhs=spk1buf[:, (t_global + 1) * BL:
                                     (t_global + 1 + nsteps) * BL],
                         start=True, stop=True)
        for k in range(nsteps):
            t = t_global + k
            mprev = m2rec[:, t * BL:(t + 1) * BL]
            mcur = m2rec[:, (t + 1) * BL:(t + 2) * BL]
            sprev = s2rec[:, t * BL:(t + 1) * BL]
            scur = s2rec[:, (t + 1) * BL:(t + 2) * BL]
            nc.vector.scalar_tensor_tensor(
                mcur, mprev, BETA2, sprev, Alu.mult, Alu.subtract)
            nc.vector.tensor_add(mcur, mcur, c2[:, k * BL:(k + 1) * BL])
            nc.vector.tensor_scalar(scur, mcur, THRESH, None, Alu.is_gt)
        t_global += nsteps

    nc.sync.dma_start(spk_o[:], s2rec[:, BL:BL + TBL])
    nc.sync.dma_start(mem_o[:], m2rec[:, BL:BL + TBL])
    stack.close()


def build_program(precision=None):
    precision = precision or PRECISION
    if precision in _PROG:
        return _PROG[precision]
    import concourse.tile as tile
    from concourse import bacc, mybir

    f32 = mybir.dt.float32
    mm_dt = {"fp32": f32, "bf16x2": mybir.dt.bfloat16}[precision]
    NS = 2 if precision == "bf16x2" else 1
    nc = bacc.Bacc("TRN2", target_bir_lowering=False, debug=False,
                   num_devices=NCORES)
    xts = [nc.dram_tensor(f"xt{j}", [KP, NS * KCH * cols], mm_dt,
                          kind="ExternalInput").ap()
           for j, cols in enumerate(COL_TILES)]
    w1f = nc.dram_tensor("w1f", [KP, NS * KCH * NH], mm_dt,
                         kind="ExternalInput").ap()
    a1 = nc.dram_tensor("a1", [KP, NH], f32, kind="ExternalInput").ap()
    w2a = nc.dram_tensor("w2a", [KP, NOUT], f32, kind="ExternalInput").ap()
    b1 = nc.dram_tensor("b1", [NH, 1], f32, kind="ExternalInput").ap()
    bet = nc.dram_tensor("bet", [NH, 1], f32, kind="ExternalInput").ap()
    s1init = nc.dram_tensor("s1init", [KP, BL + TBL], f32,
                            kind="ExternalInput").ap()
    spk_o = nc.dram_tensor("spk", [NOUT, TBL], f32, kind="ExternalOutput").ap()
    mem_o = nc.dram_tensor("mem", [NOUT, TBL], f32, kind="ExternalOutput").ap()
    aps = (xts, w1f, a1, w2a, b1, bet, s1init, spk_o, mem_o)
    with tile.TileContext(nc) as tc:
        _build_body(tc, nc, mybir, aps, precision)
    nc.compile()
    _PROG[precision] = nc
    return nc


def _mm_layout(kxn, nsplit):
    """[K=NIN, N] fp32 -> [128, nsplit*KCH*N] in matmul-ready order
    (chunk-major, hi|lo interleaved per chunk)."""
    import ml_dtypes
    n = kxn.shape[1]
    v = np.ascontiguousarray(
        kxn.reshape(KCH, KP, n).transpose(1, 0, 2))     # [128, KCH, n]
    if nsplit == 1:
        return v.reshape(KP, KCH * n)
    hi = v.astype(ml_dtypes.bfloat16)
    lo = (v - hi.astype(np.float32)).astype(ml_dtypes.bfloat16)
    out = np.empty((KP, KCH, 2, n), hi.dtype)
    out[:, :, 0, :] = hi
    out[:, :, 1, :] = lo
    return np.ascontiguousarray(out).reshape(KP, 2 * KCH * n)


def prep_inputs(x, W1, b1, beta1, V, Vb, W2, b2, precision=None):
    """Host-side shard + layout prep. Returns list of per-core input dicts."""
    precision = precision or PRECISION
    nsplit = 2 if precision == "bf16x2" else 1
    f32 = np.float32
    w1f = _mm_layout(np.ascontiguousarray(W1.T, dtype=f32), nsplit)
    a1 = np.zeros((KP, NH), f32)
    a1[:NH] = (V - THRESH * np.eye(NH, dtype=f32)).T
    a1[NH] = Vb
    w2a = np.zeros((KP, NOUT), f32)
    w2a[:NH] = W2.T
    w2a[NH] = b2
    b1a = np.ascontiguousarray(b1.reshape(NH, 1), dtype=f32)
    beta = (-np.clip(beta1, 0.0, 1.0)).astype(f32).reshape(NH, 1)  # negated
    s1init = np.zeros((KP, BL + TBL), f32)
    s1init[NH] = 1.0
    # x: [T, B, NIN] -> per-core column tiles in matmul-ready layout
    xt_full = np.ascontiguousarray(x.transpose(2, 0, 1))        # [NIN, T, B]
    col_edges = np.cumsum([0] + COL_TILES)
    in_maps = []
    for c in range(NCORES):
        xTc = np.ascontiguousarray(
            xt_full[:, :, c * BL:(c + 1) * BL]).reshape(NIN, TBL)
        m = dict(w1f=w1f, a1=a1, w2a=w2a, b1=b1a, bet=beta, s1init=s1init)
        for j, cols in enumerate(COL_TILES):
            m[f"xt{j}"] = _mm_layout(
                np.ascontiguousarray(xTc[:, col_edges[j]:col_edges[j + 1]]),
                nsplit)
        in_maps.append(m)
    return in_maps


def gather_outputs(results):
    """results: list of per-core {'spk': [NOUT, TBL], 'mem': [NOUT, TBL]}."""
    spks, mems = [], []
    for r in results:
        spks.append(np.ascontiguousarray(
            r["spk"].reshape(NOUT, T, BL).transpose(1, 2, 0)))
        mems.append(np.ascontiguousarray(
            r["mem"].reshape(NOUT, T, BL).transpose(1, 2, 0)))
    spk = np.concatenate(spks, axis=1)
    mem = np.concatenate(mems, axis=1)
    return spk.astype(np.float32), mem.astype(np.float32)


def kernel(x, W1, b1, beta1, V, Vb, W2, b2, **_run_kwargs):
    from concourse import bass_utils

    precision = _run_kwargs.pop("precision", None) or PRECISION
    nc = build_program(precision)
    in_maps = prep_inputs(np.asarray(x, np.float32), np.asarray(W1, np.float32),
                          np.asarray(b1, np.float32), np.asarray(beta1, np.float32),
                          np.asarray(V, np.float32), np.asarray(Vb, np.float32),
                          np.asarray(W2, np.float32), np.asarray(b2, np.float32),
                          precision)
    res = bass_utils.run_bass_kernel_spmd(
        nc, in_maps, core_ids=list(range(NCORES)), **_run_kwargs)
    out = gather_outputs(res.results)
    kernel.last_result = res
    return out


# revision 21
# speedup vs baseline: 1.0271x; 1.0199x over previous
"""Trainium2 Bass kernel for nn_Net_34359738709 (spiking RNN).

Model (per timestep t, reference semantics):
    cur1  = x_t @ W1.T + b1                      # [B, NH] big matmul, t-independent
    mem1  = beta1c*mem1 + cur1 + spk1 @ V.T + Vb - spk1*THRESH
    spk1  = (mem1 - THRESH > 0)
    cur2  = spk1 @ W2.T + b2
    mem2  = BETA2*mem2 + cur2 - spk2_prev*THRESH
    spk2  = (mem2 - THRESH > 0)
outputs: (spk2_rec, mem2_rec), each [T, B, NOUT]

Strategy: data-parallel over batch (B=64 -> 8 cores x 8). The x @ W1.T
matmul (21 GFLOP) is hoisted out of the time scan and computed as
cur1.T[NH, T*BL] = W1 @ x.T, accumulated over 256 K-chunks of 128 in one
PSUM bank per column tile. fp32 accuracy at bf16 speed via a hi/lo split:
x = x_hi + x_lo, W1 = w_hi + w_lo (each bf16), cur1 ~= w_hi@x_hi +
w_hi@x_lo + w_lo@x_hi (verified exact spike pattern vs fp32). Two column
tiles (34/16 timesteps): the first tile's sequential scan overlaps the
second tile's matmuls. Per scan step, layer 1 runs one augmented
128-contraction matmul (lhsT rows 0..99 = (V-I).T, row 100 = Vb, rhs =
[spk1; 1; 0]) plus three vector ops; layer 2 uses a per-burst batched
W2 matmul then a vector-only 3-op chain per step. All inputs are
host-pre-arranged into exact SBUF layouts so DMA runs long-contiguous.
"""

import sys

if "/opt/trn_rl_repo" not in sys.path:
    sys.path.insert(0, "/opt/trn_rl_repo")

import numpy as np

# Problem shapes (hardcoded per contract)
T, B, NIN, NH, NOUT = 50, 64, 32768, 100, 11
NCORES = 8
BL = B // NCORES          # 8 batch rows per core
TBL = T * BL              # 400 columns (t-major: col = t*BL + b)
KP = 128                  # contraction partition size
KCH = NIN // KP           # 256 K-chunks
COL_TILES = [256, 144]    # ncols per column tile, each % BL == 0
X_GROUPS = [2, 2, 4, 8] + [16] * 15   # K-chunks per x dma_start (sums to 256)
THRESH = 1.0
BETA2 = 0.9753

PRECISION = "bf16x2"      # "fp32" | "bf16x2"

_PROG = {}


def _build_body(tc, nc, mybir, aps, precision):
    f32 = mybir.dt.float32
    Alu = mybir.AluOpType
    mm_dt = {"fp32": f32, "bf16x2": mybir.dt.bfloat16}[precision]
    NS = 2 if precision == "bf16x2" else 1
    xts, w1f, a1, w2a, b1, bet, s1init, spk_o, mem_o = aps

    from contextlib import ExitStack

    stack = ExitStack()
    const_pool = stack.enter_context(tc.tile_pool(name="const", bufs=1))
    state_pool = stack.enter_context(tc.tile_pool(name="state", bufs=1))
    xpool = stack.enter_context(tc.tile_pool(name="xg", bufs=4))
    curpool = stack.enter_context(tc.tile_pool(name="cur", bufs=2))
    c2pool = stack.enter_context(tc.tile_pool(name="c2s", bufs=2))
    ps_a = stack.enter_context(tc.tile_pool(name="psa", bufs=2, space="PSUM"))
    ps_b = stack.enter_context(tc.tile_pool(name="psb", bufs=2, space="PSUM"))
    ps_s1 = stack.enter_context(tc.tile_pool(name="pss1", bufs=2, space="PSUM"))
    ps_c2 = stack.enter_context(tc.tile_pool(name="psc2", bufs=2, space="PSUM"))

    MAXC = max(COL_TILES)
    MAXG = max(X_GROUPS)

    # ---- small constants (issued on the scalar ring AFTER W1 streaming
    # starts; only needed by the scan, ~100us into the kernel) ----
    a1sb = const_pool.tile([KP, NH], f32)
    w2sb = const_pool.tile([KP, NOUT], f32)
    b1sb = const_pool.tile([NH, 1], f32)
    betnsb = const_pool.tile([NH, 1], f32)   # NEGATED clipped beta1
    # spk1 ring buffer: col block t+1 = spk1 after step t; rows 100..127
    # carry the [1; 0-pad] augmentation for every column (from s1init).
    spk1buf = state_pool.tile([KP, BL + TBL], f32)

    def load_consts():
        nc.scalar.dma_start(a1sb[:], a1)
        nc.scalar.dma_start(w2sb[:], w2a)
        nc.scalar.dma_start(b1sb[:], b1)
        nc.scalar.dma_start(betnsb[:], bet)
        nc.scalar.dma_start(spk1buf[:], s1init)

    # W1 resident in SBUF, exact matmul layout [128, NS*KCH*NH] (bf16 hi|lo)
    w1sb = const_pool.tile([KP, NS * KCH * NH], mm_dt)

    def w1_chunk(c, s=0):
        base = (c * NS + s) * NH
        return w1sb[:, base:base + NH]

    # ---- state ----
    mem1 = state_pool.tile([NH, BL], f32)
    nc.vector.memset(mem1[:], 0.0)
    m2rec = state_pool.tile([NOUT, BL + TBL], f32)
    s2rec = state_pool.tile([NOUT, BL + TBL], f32)
    nc.vector.memset(m2rec[:, 0:BL], 0.0)
    nc.vector.memset(s2rec[:, 0:BL], 0.0)

    tmpneg = state_pool.tile([NH, BL], f32)

    t_global = 0
    for j, cols in enumerate(COL_TILES):
        xt = xts[j]           # [128, NS*KCH*cols] dram, matmul-ready
        # psA accumulates [w_hi@x_hi | w_hi@x_lo] (N=2*cols); psB w_lo@x_hi
        psa = ps_a.tile([NH, 2 * MAXC], f32)
        psb = ps_b.tile([NH, MAXC], f32)
        c0 = 0
        for g, gch in enumerate(X_GROUPS):
            if j == 0:
                # stream the matching W1 chunk range on the scalar ring
                w0, w1n = c0 * NS * NH, (c0 + gch) * NS * NH
                nc.scalar.dma_start(w1sb[:, w0:w1n], w1f[:, w0:w1n])
                if g == 4:
                    load_consts()
            xg = xpool.tile([KP, NS * MAXG * MAXC], mm_dt)
            gsz = NS * gch * cols
            dma_eng = (nc.sync, nc.gpsimd)[g % 2]
            dma_eng.dma_start(xg[:, :gsz], xt[:, c0 * NS * cols:(c0 + gch) * NS * cols])

            if NS == 1:
                for ci in range(gch):
                    c = c0 + ci
                    nc.tensor.matmul(
                        psa[:, :cols], lhsT=w1_chunk(c),
                        rhs=xg[:, ci * cols:(ci + 1) * cols],
                        start=(c == 0), stop=(c == KCH - 1))
            else:
                # hi/lo split: one MM covers hh|hl (concat cols), one lh.
                # All w_hi MMs of the group first, then all w_lo MMs, so the
                # PSUM write bank switches once per group, not per chunk.
                for ci in range(gch):
                    c = c0 + ci
                    nc.tensor.matmul(
                        psa[:, :2 * cols], lhsT=w1_chunk(c, 0),
                        rhs=xg[:, ci * 2 * cols:(ci + 1) * 2 * cols],
                        start=(c == 0), stop=(c == KCH - 1))
                for ci in range(gch):
                    c = c0 + ci
                    nc.tensor.matmul(
                        psb[:, :cols], lhsT=w1_chunk(c, 1),
                        rhs=xg[:, ci * 2 * cols:ci * 2 * cols + cols],
                        start=(c == 0), stop=(c == KCH - 1))
            c0 += gch
        cur = curpool.tile([NH, MAXC], f32)
        if NS == 1:
            nc.vector.tensor_scalar_add(cur[:, :cols], psa[:, :cols], b1sb[:, 0:1])
        else:
            # cur = psa_hh + b1, += psa_hl, += psb (one PSUM read per op)
            nc.vector.tensor_scalar_add(cur[:, :cols], psa[:, :cols],
                                        b1sb[:, 0:1])
            nc.vector.tensor_add(cur[:, :cols], cur[:, :cols],
                                 psa[:, cols:2 * cols])
            nc.vector.tensor_add(cur[:, :cols], cur[:, :cols], psb[:, :cols])

        # ---- scan for this tile's timesteps ----
        # Layer 1 (PE+DVE critical loop):
        #   tmpneg = -beta*mem1 - cur_t   (independent of the V matmul)
        #   spk1   = (rec - 1) > tmpneg   (single fused op after the matmul)
        #   mem1   = rec - tmpneg
        # Layer 2 (off the critical path): per 8 steps one batched W2 matmul
        # (PE, interleaved), PSUM->SBUF copy on ScalarE, then a 3-op chain
        # per step on GpSimd.
        nsteps = cols // BL
        nc.vector.scalar_tensor_tensor(
            tmpneg[:], mem1[:], betnsb[:, 0:1], cur[:, 0:BL],
            Alu.mult, Alu.subtract)
        for k in range(nsteps):
            t = t_global + k
            rec = ps_s1.tile([NH, BL], f32)
            nc.tensor.matmul(rec[:], lhsT=a1sb[:, :],
                             rhs=spk1buf[:, t * BL:(t + 1) * BL],
                             start=True, stop=True)
            nc.vector.scalar_tensor_tensor(
                spk1buf[0:NH, (t + 1) * BL:(t + 2) * BL], rec[:],
                THRESH, tmpneg[:], Alu.subtract, Alu.is_gt)
            nc.vector.tensor_sub(mem1[:], rec[:], tmpneg[:])
            if k + 1 < nsteps:
                nc.vector.scalar_tensor_tensor(
                    tmpneg[:], mem1[:], betnsb[:, 0:1],
                    cur[:, (k + 1) * BL:(k + 2) * BL], Alu.mult, Alu.subtract)
            if k + 1 == nsteps or (k + 1) % 8 == 0:
                k0 = (k // 8) * 8
                kn = k + 1 - k0
                c2 = ps_c2.tile([NOUT, 8 * BL], f32)
                nc.tensor.matmul(c2[:, :kn * BL], lhsT=w2sb[:, :],
                                 rhs=spk1buf[:, (t_global + k0 + 1) * BL:
                                             (t_global + k0 + 1 + kn) * BL],
                                 start=True, stop=True)
                for kk in range(k0, k0 + kn):
                    tt = t_global + kk
                    mprev = m2rec[:, tt * BL:(tt + 1) * BL]
                    mcur = m2rec[:, (tt + 1) * BL:(tt + 2) * BL]
                    sprev = s2rec[:, tt * BL:(tt + 1) * BL]
                    scur = s2rec[:, (tt + 1) * BL:(tt + 2) * BL]
                    nc.vector.scalar_tensor_tensor(
                        mcur, mprev, BETA2, sprev, Alu.mult, Alu.subtract)
                    nc.vector.tensor_add(
                        mcur, mcur, c2[:, (kk - k0) * BL:(kk - k0 + 1) * BL])
                    nc.vector.tensor_scalar(scur, mcur, THRESH, None, Alu.is_gt)
        t_global += nsteps

    nc.sync.dma_start(spk_o[:], s2rec[:, BL:BL + TBL])
    nc.sync.dma_start(mem_o[:], m2rec[:, BL:BL + TBL])
    stack.close()


def build_program(precision=None):
    precision = precision or PRECISION
    if precision in _PROG:
        return _PROG[precision]
    import concourse.tile as tile
    from concourse import bacc, mybir

    f32 = mybir.dt.float32
    mm_dt = {"fp32": f32, "bf16x2": mybir.dt.bfloat16}[precision]
    NS = 2 if precision == "bf16x2" else 1
    nc = bacc.Bacc("TRN2", target_bir_lowering=False, debug=False,
                   num_devices=NCORES)
    xts = [nc.dram_tensor(f"xt{j}", [KP, NS * KCH * cols], mm_dt,
                          kind="ExternalInput").ap()
           for j, cols in enumerate(COL_TILES)]
    w1f = nc.dram_tensor("w1f", [KP, NS * KCH * NH], mm_dt,
                         kind="ExternalInput").ap()
    a1 = nc.dram_tensor("a1", [KP, NH], f32, kind="ExternalInput").ap()
    w2a = nc.dram_tensor("w2a", [KP, NOUT], f32, kind="ExternalInput").ap()
    b1 = nc.dram_tensor("b1", [NH, 1], f32, kind="ExternalInput").ap()
    bet = nc.dram_tensor("bet", [NH, 1], f32, kind="ExternalInput").ap()
    s1init = nc.dram_tensor("s1init", [KP, BL + TBL], f32,
                            kind="ExternalInput").ap()
    spk_o = nc.dram_tensor("spk", [NOUT, TBL], f32, kind="ExternalOutput").ap()
    mem_o = nc.dram_tensor("mem", [NOUT, TBL], f32, kind="ExternalOutput").ap()
    aps = (xts, w1f, a1, w2a, b1, bet, s1init, spk_o, mem_o)
    with tile.TileContext(nc) as tc:
        _build_body(tc, nc, mybir, aps, precision)
    nc.compile()
    _PROG[precision] = nc
    return nc


def _mm_layout(kxn, nsplit):
    """[K=NIN, N] fp32 -> [128, nsplit*KCH*N] in matmul-ready order
    (chunk-major, hi|lo interleaved per chunk)."""
    import ml_dtypes
    n = kxn.shape[1]
    v = np.ascontiguousarray(
        kxn.reshape(KCH, KP, n).transpose(1, 0, 2))     # [128, KCH, n]
    if nsplit == 1:
        return v.reshape(KP, KCH * n)
    hi = v.astype(ml_dtypes.bfloat16)
    lo = (v - hi.astype(np.float32)).astype(ml_dtypes.bfloat16)
    out = np.empty((KP, KCH, 2, n), hi.dtype)
    out[:, :, 0, :] = hi
    out[:, :, 1, :] = lo
    return np.ascontiguousarray(out).reshape(KP, 2 * KCH * n)


def prep_inputs(x, W1, b1, beta1, V, Vb, W2, b2, precision=None):
    """Host-side shard + layout prep. Returns list of per-core input dicts."""
    precision = precision or PRECISION
    nsplit = 2 if precision == "bf16x2" else 1
    f32 = np.float32
    w1f = _mm_layout(np.ascontiguousarray(W1.T, dtype=f32), nsplit)
    a1 = np.zeros((KP, NH), f32)
    a1[:NH] = (V - THRESH * np.eye(NH, dtype=f32)).T
    a1[NH] = Vb
    w2a = np.zeros((KP, NOUT), f32)
    w2a[:NH] = W2.T
    w2a[NH] = b2
    b1a = np.ascontiguousarray(b1.reshape(NH, 1), dtype=f32)
    beta = (-np.clip(beta1, 0.0, 1.0)).astype(f32).reshape(NH, 1)  # negated
    s1init = np.zeros((KP, BL + TBL), f32)
    s1init[NH] = 1.0
    # x: [T, B, NIN] -> per-core column tiles in matmul-ready layout
    xt_full = np.ascontiguousarray(x.transpose(2, 0, 1))        # [NIN, T, B]
    col_edges = np.cumsum([0] + COL_TILES)
    in_maps = []
    for c in range(NCORES):
        xTc = np.ascontiguousarray(
            xt_full[:, :, c * BL:(c + 1) * BL]).reshape(NIN, TBL)
        m = dict(w1f=w1f, a1=a1, w2a=w2a, b1=b1a, bet=beta, s1init=s1init)
        for j, cols in enumerate(COL_TILES):
            m[f"xt{j}"] = _mm_layout(
                np.ascontiguousarray(xTc[:, col_edges[j]:col_edges[j + 1]]),
                nsplit)
        in_maps.append(m)
    return in_maps


def gather_outputs(results):
    """results: list of per-core {'spk': [NOUT, TBL], 'mem': [NOUT, TBL]}."""
    spks, mems = [], []
    for r in results:
        spks.append(np.ascontiguousarray(
            r["spk"].reshape(NOUT, T, BL).transpose(1, 2, 0)))
        mems.append(np.ascontiguousarray(
            r["mem"].reshape(NOUT, T, BL).transpose(1, 2, 0)))
    spk = np.concatenate(spks, axis=1)
    mem = np.concatenate(mems, axis=1)
    return spk.astype(np.float32), mem.astype(np.float32)


def kernel(x, W1, b1, beta1, V, Vb, W2, b2, **_run_kwargs):
    from concourse import bass_utils

    precision = _run_kwargs.pop("precision", None) or PRECISION
    nc = build_program(precision)
    in_maps = prep_inputs(np.asarray(x, np.float32), np.asarray(W1, np.float32),
                          np.asarray(b1, np.float32), np.asarray(beta1, np.float32),
                          np.asarray(V, np.float32), np.asarray(Vb, np.float32),
                          np.asarray(W2, np.float32), np.asarray(b2, np.float32),
                          precision)
    res = bass_utils.run_bass_kernel_spmd(
        nc, in_maps, core_ids=list(range(NCORES)), **_run_kwargs)
    out = gather_outputs(res.results)
    kernel.last_result = res
    return out


# revision 23
# speedup vs baseline: 1.0426x; 1.0150x over previous
"""Trainium2 Bass kernel for nn_Net_34359738709 (spiking RNN).

Model (per timestep t, reference semantics):
    cur1  = x_t @ W1.T + b1                      # [B, NH] big matmul, t-independent
    mem1  = beta1c*mem1 + cur1 + spk1 @ V.T + Vb - spk1*THRESH
    spk1  = (mem1 - THRESH > 0)
    cur2  = spk1 @ W2.T + b2
    mem2  = BETA2*mem2 + cur2 - spk2_prev*THRESH
    spk2  = (mem2 - THRESH > 0)
outputs: (spk2_rec, mem2_rec), each [T, B, NOUT]

Strategy: data-parallel over batch (B=64 -> 8 cores x 8). The x @ W1.T
matmul (21 GFLOP) is hoisted out of the time scan and computed as
cur1.T[NH, T*BL] = W1 @ x.T, accumulated over 256 K-chunks of 128 in one
PSUM bank per column tile. fp32 accuracy at bf16 speed via a hi/lo split:
x = x_hi + x_lo, W1 = w_hi + w_lo (each bf16), cur1 ~= w_hi@x_hi +
w_hi@x_lo + w_lo@x_hi (verified exact spike pattern vs fp32). Two column
tiles (34/16 timesteps): the first tile's sequential scan overlaps the
second tile's matmuls. Per scan step, layer 1 runs one augmented
128-contraction matmul (lhsT rows 0..99 = (V-I).T, row 100 = Vb, rhs =
[spk1; 1; 0]) plus three vector ops; layer 2 uses a per-burst batched
W2 matmul then a vector-only 3-op chain per step. All inputs are
host-pre-arranged into exact SBUF layouts so DMA runs long-contiguous.
"""

import sys

if "/opt/trn_rl_repo" not in sys.path:
    sys.path.insert(0, "/opt/trn_rl_repo")

import numpy as np

# Problem shapes (hardcoded per contract)
T, B, NIN, NH, NOUT = 50, 64, 32768, 100, 11
NCORES = 8
BL = B // NCORES          # 8 batch rows per core
TBL = T * BL              # 400 columns (t-major: col = t*BL + b)
KP = 128                  # contraction partition size
KCH = NIN // KP           # 256 K-chunks
COL_TILES = [256, 144]    # ncols per column tile, each % BL == 0
X_GROUPS = [2, 2, 4, 8] + [16] * 15   # K-chunks per x dma_start (sums to 256)
THRESH = 1.0
BETA2 = 0.9753

PRECISION = "bf16x2"      # "fp32" | "bf16x2"

_PROG = {}


def _build_body(tc, nc, mybir, aps, precision):
    f32 = mybir.dt.float32
    Alu = mybir.AluOpType
    mm_dt = {"fp32": f32, "bf16x2": mybir.dt.bfloat16}[precision]
    NS = 2 if precision == "bf16x2" else 1
    xts, w1f, a1, w2a, b1, bet, s1init, spk_o, mem_o = aps

    from contextlib import ExitStack

    stack = ExitStack()
    const_pool = stack.enter_context(tc.tile_pool(name="const", bufs=1))
    state_pool = stack.enter_context(tc.tile_pool(name="state", bufs=1))
    xpool = stack.enter_context(tc.tile_pool(name="xg", bufs=4))
    curpool = stack.enter_context(tc.tile_pool(name="cur", bufs=2))
    c2pool = stack.enter_context(tc.tile_pool(name="c2s", bufs=2))
    ps_a = stack.enter_context(tc.tile_pool(name="psa", bufs=2, space="PSUM"))
    ps_s1 = stack.enter_context(tc.tile_pool(name="pss1", bufs=2, space="PSUM"))
    ps_c2 = stack.enter_context(tc.tile_pool(name="psc2", bufs=2, space="PSUM"))

    MAXC = max(COL_TILES)
    MAXG = max(X_GROUPS)

    # ---- small constants (issued on the scalar ring AFTER W1 streaming
    # starts; only needed by the scan, ~100us into the kernel) ----
    a1sb = const_pool.tile([KP, NH], f32)
    w2sb = const_pool.tile([KP, NOUT], f32)
    b1sb = const_pool.tile([NH, 1], f32)
    betnsb = const_pool.tile([NH, 1], f32)   # NEGATED clipped beta1
    # spk1 ring buffer: col block t+1 = spk1 after step t; rows 100..127
    # carry the [1; 0-pad] augmentation for every column (from s1init).
    spk1buf = state_pool.tile([KP, BL + TBL], f32)

    def load_consts():
        nc.scalar.dma_start(a1sb[:], a1)
        nc.scalar.dma_start(w2sb[:], w2a)
        nc.scalar.dma_start(b1sb[:], b1)
        nc.scalar.dma_start(betnsb[:], bet)
        nc.scalar.dma_start(spk1buf[:], s1init)

    # W1 resident in SBUF, exact matmul layout [128, NS*KCH*NH] (bf16 hi|lo)
    w1sb = const_pool.tile([KP, NS * KCH * NH], mm_dt)

    def w1_chunk(c, s=0):
        base = (c * NS + s) * NH
        return w1sb[:, base:base + NH]

    # ---- state ----
    mem1 = state_pool.tile([NH, BL], f32)
    nc.vector.memset(mem1[:], 0.0)
    m2rec = state_pool.tile([NOUT, BL + TBL], f32)
    s2rec = state_pool.tile([NOUT, BL + TBL], f32)
    nc.vector.memset(m2rec[:, 0:BL], 0.0)
    nc.vector.memset(s2rec[:, 0:BL], 0.0)

    tmpneg = state_pool.tile([NH, BL], f32)

    t_global = 0
    for j, cols in enumerate(COL_TILES):
        xt = xts[j]           # [128, NS*KCH*cols] dram, matmul-ready
        psa = ps_a.tile([NH, MAXC], f32)
        c0 = 0
        for g, gch in enumerate(X_GROUPS):
            if j == 0:
                # stream the matching W1 chunk range on the scalar ring
                w0, w1n = c0 * NS * NH, (c0 + gch) * NS * NH
                nc.scalar.dma_start(w1sb[:, w0:w1n], w1f[:, w0:w1n])
                if g == 4:
                    load_consts()
            xg = xpool.tile([KP, NS * MAXG * MAXC], mm_dt)
            gsz = NS * gch * cols
            dma_eng = (nc.sync, nc.gpsimd)[g % 2]
            dma_eng.dma_start(xg[:, :gsz], xt[:, c0 * NS * cols:(c0 + gch) * NS * cols])

            if NS == 1:
                for ci in range(gch):
                    c = c0 + ci
                    nc.tensor.matmul(
                        psa[:, :cols], lhsT=w1_chunk(c),
                        rhs=xg[:, ci * cols:(ci + 1) * cols],
                        start=(c == 0), stop=(c == KCH - 1))
            else:
                # hi/lo split: hh + hl + lh into one PSUM bank
                for ci in range(gch):
                    c = c0 + ci
                    nc.tensor.matmul(
                        psa[:, :cols], lhsT=w1_chunk(c, 0),
                        rhs=xg[:, ci * 2 * cols:ci * 2 * cols + cols],
                        start=(c == 0), stop=False)
                    nc.tensor.matmul(
                        psa[:, :cols], lhsT=w1_chunk(c, 0),
                        rhs=xg[:, ci * 2 * cols + cols:(ci + 1) * 2 * cols],
                        start=False, stop=False)
                    nc.tensor.matmul(
                        psa[:, :cols], lhsT=w1_chunk(c, 1),
                        rhs=xg[:, ci * 2 * cols:ci * 2 * cols + cols],
                        start=False, stop=(c == KCH - 1))
            c0 += gch
        cur = curpool.tile([NH, MAXC], f32)
        nc.vector.tensor_scalar_add(cur[:, :cols], psa[:, :cols], b1sb[:, 0:1])

        # ---- scan for this tile's timesteps ----
        # Layer 1 (PE+DVE critical loop):
        #   tmpneg = -beta*mem1 - cur_t   (independent of the V matmul)
        #   spk1   = (rec - 1) > tmpneg   (single fused op after the matmul)
        #   mem1   = rec - tmpneg
        # Layer 2 (off the critical path): per 8 steps one batched W2 matmul
        # (PE, interleaved), PSUM->SBUF copy on ScalarE, then a 3-op chain
        # per step on GpSimd.
        nsteps = cols // BL
        nc.vector.scalar_tensor_tensor(
            tmpneg[:], mem1[:], betnsb[:, 0:1], cur[:, 0:BL],
            Alu.mult, Alu.subtract)
        for k in range(nsteps):
            t = t_global + k
            rec = ps_s1.tile([NH, BL], f32)
            nc.tensor.matmul(rec[:], lhsT=a1sb[:, :],
                             rhs=spk1buf[:, t * BL:(t + 1) * BL],
                             start=True, stop=True)
            nc.vector.scalar_tensor_tensor(
                spk1buf[0:NH, (t + 1) * BL:(t + 2) * BL], rec[:],
                THRESH, tmpneg[:], Alu.subtract, Alu.is_gt)
            nc.vector.tensor_sub(mem1[:], rec[:], tmpneg[:])
            if k + 1 < nsteps:
                nc.vector.scalar_tensor_tensor(
                    tmpneg[:], mem1[:], betnsb[:, 0:1],
                    cur[:, (k + 1) * BL:(k + 2) * BL], Alu.mult, Alu.subtract)
            if k + 1 == nsteps or (k + 1) % 8 == 0:
                k0 = (k // 8) * 8
                kn = k + 1 - k0
                c2 = ps_c2.tile([NOUT, 8 * BL], f32)
                nc.tensor.matmul(c2[:, :kn * BL], lhsT=w2sb[:, :],
                                 rhs=spk1buf[:, (t_global + k0 + 1) * BL:
                                             (t_global + k0 + 1 + kn) * BL],
                                 start=True, stop=True)
                for kk in range(k0, k0 + kn):
                    tt = t_global + kk
                    mprev = m2rec[:, tt * BL:(tt + 1) * BL]
                    mcur = m2rec[:, (tt + 1) * BL:(tt + 2) * BL]
                    sprev = s2rec[:, tt * BL:(tt + 1) * BL]
                    scur = s2rec[:, (tt + 1) * BL:(tt + 2) * BL]
                    nc.vector.scalar_tensor_tensor(
                        mcur, mprev, BETA2, sprev, Alu.mult, Alu.subtract)
                    nc.vector.tensor_add(
                        mcur, mcur, c2[:, (kk - k0) * BL:(kk - k0 + 1) * BL])
                    nc.vector.tensor_scalar(scur, mcur, THRESH, None, Alu.is_gt)
        t_global += nsteps

    nc.sync.dma_start(spk_o[:], s2rec[:, BL:BL + TBL])
    nc.sync.dma_start(mem_o[:], m2rec[:, BL:BL + TBL])
    stack.close()


def build_program(precision=None):
    precision = precision or PRECISION
    if precision in _PROG:
        return _PROG[precision]
    import concourse.tile as tile
    from concourse import bacc, mybir

    f32 = mybir.dt.float32
    mm_dt = {"fp32": f32, "bf16x2": mybir.dt.bfloat16}[precision]
    NS = 2 if precision == "bf16x2" else 1
    nc = bacc.Bacc("TRN2", target_bir_lowering=False, debug=False,
                   num_devices=NCORES)
    xts = [nc.dram_tensor(f"xt{j}", [KP, NS * KCH * cols], mm_dt,
                          kind="ExternalInput").ap()
           for j, cols in enumerate(COL_TILES)]
    w1f = nc.dram_tensor("w1f", [KP, NS * KCH * NH], mm_dt,
                         kind="ExternalInput").ap()
    a1 = nc.dram_tensor("a1", [KP, NH], f32, kind="ExternalInput").ap()
    w2a = nc.dram_tensor("w2a", [KP, NOUT], f32, kind="ExternalInput").ap()
    b1 = nc.dram_tensor("b1", [NH, 1], f32, kind="ExternalInput").ap()
    bet = nc.dram_tensor("bet", [NH, 1], f32, kind="ExternalInput").ap()
    s1init = nc.dram_tensor("s1init", [KP, BL + TBL], f32,
                            kind="ExternalInput").ap()
    spk_o = nc.dram_tensor("spk", [NOUT, TBL], f32, kind="ExternalOutput").ap()
    mem_o = nc.dram_tensor("mem", [NOUT, TBL], f32, kind="ExternalOutput").ap()
    aps = (xts, w1f, a1, w2a, b1, bet, s1init, spk_o, mem_o)
    with tile.TileContext(nc) as tc:
        _build_body(tc, nc, mybir, aps, precision)
    nc.compile()
    _PROG[precision] = nc
    return nc


def _mm_layout(kxn, nsplit):
    """[K=NIN, N] fp32 -> [128, nsplit*KCH*N] in matmul-ready order
    (chunk-major, hi|lo interleaved per chunk)."""
    import ml_dtypes
    n = kxn.shape[1]
    v = np.ascontiguousarray(
        kxn.reshape(KCH, KP, n).transpose(1, 0, 2))     # [128, KCH, n]
    if nsplit == 1:
        return v.reshape(KP, KCH * n)
    hi = v.astype(ml_dtypes.bfloat16)
    lo = (v - hi.astype(np.float32)).astype(ml_dtypes.bfloat16)
    out = np.empty((KP, KCH, 2, n), hi.dtype)
    out[:, :, 0, :] = hi
    out[:, :, 1, :] = lo
    return np.ascontiguousarray(out).reshape(KP, 2 * KCH * n)


def prep_inputs(x, W1, b1, beta1, V, Vb, W2, b2, precision=None):
    """Host-side shard + layout prep. Returns list of per-core input dicts."""
    precision = precision or PRECISION
    nsplit = 2 if precision == "bf16x2" else 1
    f32 = np.float32
    w1f = _mm_layout(np.ascontiguousarray(W1.T, dtype=f32), nsplit)
    a1 = np.zeros((KP, NH), f32)
    a1[:NH] = (V - THRESH * np.eye(NH, dtype=f32)).T
    a1[NH] = Vb
    w2a = np.zeros((KP, NOUT), f32)
    w2a[:NH] = W2.T
    w2a[NH] = b2
    b1a = np.ascontiguousarray(b1.reshape(NH, 1), dtype=f32)
    beta = (-np.clip(beta1, 0.0, 1.0)).astype(f32).reshape(NH, 1)  # negated
    s1init = np.zeros((KP, BL + TBL), f32)
    s1init[NH] = 1.0
    # x: [T, B, NIN] -> per-core column tiles in matmul-ready layout
    xt_full = np.ascontiguousarray(x.transpose(2, 0, 1))        # [NIN, T, B]
    col_edges = np.cumsum([0] + COL_TILES)
    in_maps = []
    for c in range(NCORES):
        xTc = np.ascontiguousarray(
            xt_full[:, :, c * BL:(c + 1) * BL]).reshape(NIN, TBL)
        m = dict(w1f=w1f, a1=a1, w2a=w2a, b1=b1a, bet=beta, s1init=s1init)
        for j, cols in enumerate(COL_TILES):
            m[f"xt{j}"] = _mm_layout(
                np.ascontiguousarray(xTc[:, col_edges[j]:col_edges[j + 1]]),
                nsplit)
        in_maps.append(m)
    return in_maps


def gather_outputs(results):
    """results: list of per-core {'spk': [NOUT, TBL], 'mem': [NOUT, TBL]}."""
    spks, mems = [], []
    for r in results:
        spks.append(np.ascontiguousarray(
            r["spk"].reshape(NOUT, T, BL).transpose(1, 2, 0)))
        mems.append(np.ascontiguousarray(
            r["mem"].reshape(NOUT, T, BL).transpose(1, 2, 0)))
    spk = np.concatenate(spks, axis=1)
    mem = np.concatenate(mems, axis=1)
    return spk.astype(np.float32), mem.astype(np.float32)


def kernel(x, W1, b1, beta1, V, Vb, W2, b2, **_run_kwargs):
    from concourse import bass_utils

    precision = _run_kwargs.pop("precision", None) or PRECISION
    nc = build_program(precision)
    in_maps = prep_inputs(np.asarray(x, np.float32), np.asarray(W1, np.float32),
                          np.asarray(b1, np.float32), np.asarray(beta1, np.float32),
                          np.asarray(V, np.float32), np.asarray(Vb, np.float32),
                          np.asarray(W2, np.float32), np.asarray(b2, np.float32),
                          precision)
    res = bass_utils.run_bass_kernel_spmd(
        nc, in_maps, core_ids=list(range(NCORES)), **_run_kwargs)
    out = gather_outputs(res.results)
    kernel.last_result = res
    return out


# revision 24
# speedup vs baseline: 1.0839x; 1.0396x over previous
"""Trainium2 Bass kernel for nn_Net_34359738709 (spiking RNN).

Model (per timestep t, reference semantics):
    cur1  = x_t @ W1.T + b1                      # [B, NH] big matmul, t-independent
    mem1  = beta1c*mem1 + cur1 + spk1 @ V.T + Vb - spk1*THRESH
    spk1  = (mem1 - THRESH > 0)
    cur2  = spk1 @ W2.T + b2
    mem2  = BETA2*mem2 + cur2 - spk2_prev*THRESH
    spk2  = (mem2 - THRESH > 0)
outputs: (spk2_rec, mem2_rec), each [T, B, NOUT]

Strategy: data-parallel over batch (B=64 -> 8 cores x 8). The x @ W1.T
matmul (21 GFLOP) is hoisted out of the time scan and computed as
cur1.T[NH, T*BL] = W1 @ x.T, accumulated over 256 K-chunks of 128 in one
PSUM bank per column tile. fp32 accuracy at bf16 speed via a hi/lo split:
x = x_hi + x_lo, W1 = w_hi + w_lo (each bf16), cur1 ~= w_hi@x_hi +
w_hi@x_lo + w_lo@x_hi (verified exact spike pattern vs fp32). Two column
tiles (34/16 timesteps): the first tile's sequential scan overlaps the
second tile's matmuls. Per scan step, layer 1 runs one augmented
128-contraction matmul (lhsT rows 0..99 = (V-I).T, row 100 = Vb, rhs =
[spk1; 1; 0]) plus three vector ops; layer 2 uses a per-burst batched
W2 matmul then a vector-only 3-op chain per step. All inputs are
host-pre-arranged into exact SBUF layouts so DMA runs long-contiguous.
"""

import sys

if "/opt/trn_rl_repo" not in sys.path:
    sys.path.insert(0, "/opt/trn_rl_repo")

import numpy as np

# Problem shapes (hardcoded per contract)
T, B, NIN, NH, NOUT = 50, 64, 32768, 100, 11
NCORES = 8
BL = B // NCORES          # 8 batch rows per core
TBL = T * BL              # 400 columns (t-major: col = t*BL + b)
KP = 128                  # contraction partition size
KCH = NIN // KP           # 256 K-chunks
COL_TILES = [256, 144]    # ncols per column tile, each % BL == 0
X_GROUPS = [2, 2, 4, 8] + [16] * 15   # K-chunks per x dma_start (sums to 256)
THRESH = 1.0
BETA2 = 0.9753

PRECISION = "bf16x2"      # "fp32" | "bf16x2"

_PROG = {}


def _build_body(tc, nc, mybir, aps, precision):
    f32 = mybir.dt.float32
    Alu = mybir.AluOpType
    mm_dt = {"fp32": f32, "bf16x2": mybir.dt.bfloat16}[precision]
    NS = 2 if precision == "bf16x2" else 1
    xts, w1f, a1, w2a, b1, bet, s1init, spk_o, mem_o = aps

    from contextlib import ExitStack

    stack = ExitStack()
    const_pool = stack.enter_context(tc.tile_pool(name="const", bufs=1))
    state_pool = stack.enter_context(tc.tile_pool(name="state", bufs=1))
    xpool = stack.enter_context(tc.tile_pool(name="xg", bufs=4))
    curpool = stack.enter_context(tc.tile_pool(name="cur", bufs=2))
    c2pool = stack.enter_context(tc.tile_pool(name="c2s", bufs=2))
    ps_a = stack.enter_context(tc.tile_pool(name="psa", bufs=2, space="PSUM"))
    ps_s1 = stack.enter_context(tc.tile_pool(name="pss1", bufs=2, space="PSUM"))
    ps_c2 = stack.enter_context(tc.tile_pool(name="psc2", bufs=2, space="PSUM"))

    MAXC = max(COL_TILES)
    MAXG = max(X_GROUPS)

    # ---- small constants (issued on the scalar ring AFTER W1 streaming
    # starts; only needed by the scan, ~100us into the kernel) ----
    a1sb = const_pool.tile([KP, NH], f32)
    w2sb = const_pool.tile([KP, NOUT], f32)
    b1sb = const_pool.tile([NH, 1], f32)
    betnsb = const_pool.tile([NH, 1], f32)   # NEGATED clipped beta1
    # spk1 ring buffer: col block t+1 = spk1 after step t; rows 100..127
    # carry the [1; 0-pad] augmentation for every column (from s1init).
    spk1buf = state_pool.tile([KP, BL + TBL], f32)

    def load_consts():
        nc.scalar.dma_start(a1sb[:], a1)
        nc.scalar.dma_start(w2sb[:], w2a)
        nc.scalar.dma_start(b1sb[:], b1)
        nc.scalar.dma_start(betnsb[:], bet)
        nc.scalar.dma_start(spk1buf[:], s1init)

    # W1 resident in SBUF, exact matmul layout [128, NS*KCH*NH] (bf16 hi|lo)
    w1sb = const_pool.tile([KP, NS * KCH * NH], mm_dt)

    def w1_chunk(c, s=0):
        base = (c * NS + s) * NH
        return w1sb[:, base:base + NH]

    # ---- state ----
    mem1 = state_pool.tile([NH, BL], f32)
    nc.vector.memset(mem1[:], 0.0)
    m2rec = state_pool.tile([NOUT, BL + TBL], f32)
    s2rec = state_pool.tile([NOUT, BL + TBL], f32)
    nc.vector.memset(m2rec[:, 0:BL], 0.0)
    nc.vector.memset(s2rec[:, 0:BL], 0.0)

    tmpneg = state_pool.tile([NH, BL], f32)

    t_global = 0
    for j, cols in enumerate(COL_TILES):
        xt = xts[j]           # [128, NS*KCH*cols] dram, matmul-ready
        psa = ps_a.tile([NH, MAXC], f32)
        c0 = 0
        for g, gch in enumerate(X_GROUPS):
            if j == 0:
                # stream the matching W1 chunk range on the scalar ring
                w0, w1n = c0 * NS * NH, (c0 + gch) * NS * NH
                nc.scalar.dma_start(w1sb[:, w0:w1n], w1f[:, w0:w1n])
                if g == 4:
                    load_consts()
            xg = xpool.tile([KP, NS * MAXG * MAXC], mm_dt)
            gsz = NS * gch * cols
            nc.sync.dma_start(xg[:, :gsz], xt[:, c0 * NS * cols:(c0 + gch) * NS * cols])

            if NS == 1:
                for ci in range(gch):
                    c = c0 + ci
                    nc.tensor.matmul(
                        psa[:, :cols], lhsT=w1_chunk(c),
                        rhs=xg[:, ci * cols:(ci + 1) * cols],
                        start=(c == 0), stop=(c == KCH - 1))
            else:
                # hi/lo split: hh + hl + lh into one PSUM bank
                for ci in range(gch):
                    c = c0 + ci
                    nc.tensor.matmul(
                        psa[:, :cols], lhsT=w1_chunk(c, 0),
                        rhs=xg[:, ci * 2 * cols:ci * 2 * cols + cols],
                        start=(c == 0), stop=False)
                    nc.tensor.matmul(
                        psa[:, :cols], lhsT=w1_chunk(c, 0),
                        rhs=xg[:, ci * 2 * cols + cols:(ci + 1) * 2 * cols],
                        start=False, stop=False)
                    nc.tensor.matmul(
                        psa[:, :cols], lhsT=w1_chunk(c, 1),
                        rhs=xg[:, ci * 2 * cols:ci * 2 * cols + cols],
                        start=False, stop=(c == KCH - 1))
            c0 += gch
        cur = curpool.tile([NH, MAXC], f32)
        nc.vector.tensor_scalar_add(cur[:, :cols], psa[:, :cols], b1sb[:, 0:1])

        # ---- scan for this tile's timesteps ----
        # Layer 1 (PE+DVE critical loop):
        #   tmpneg = -beta*mem1 - cur_t   (independent of the V matmul)
        #   spk1   = (rec - 1) > tmpneg   (single fused op after the matmul)
        #   mem1   = rec - tmpneg
        # Layer 2 (off the critical path): per 8 steps one batched W2 matmul
        # (PE, interleaved), PSUM->SBUF copy on ScalarE, then a 3-op chain
        # per step on GpSimd.
        nsteps = cols // BL
        nc.vector.scalar_tensor_tensor(
            tmpneg[:], mem1[:], betnsb[:, 0:1], cur[:, 0:BL],
            Alu.mult, Alu.subtract)
        for k in range(nsteps):
            t = t_global + k
            rec = ps_s1.tile([NH, BL], f32)
            nc.tensor.matmul(rec[:], lhsT=a1sb[:, :],
                             rhs=spk1buf[:, t * BL:(t + 1) * BL],
                             start=True, stop=True)
            nc.vector.scalar_tensor_tensor(
                spk1buf[0:NH, (t + 1) * BL:(t + 2) * BL], rec[:],
                THRESH, tmpneg[:], Alu.subtract, Alu.is_gt)
            nc.vector.tensor_sub(mem1[:], rec[:], tmpneg[:])
            if k + 1 < nsteps:
                nc.vector.scalar_tensor_tensor(
                    tmpneg[:], mem1[:], betnsb[:, 0:1],
                    cur[:, (k + 1) * BL:(k + 2) * BL], Alu.mult, Alu.subtract)
            if k + 1 == nsteps or (k + 1) % 8 == 0:
                k0 = (k // 8) * 8
                kn = k + 1 - k0
                c2 = ps_c2.tile([NOUT, 8 * BL], f32)
                nc.tensor.matmul(c2[:, :kn * BL], lhsT=w2sb[:, :],
                                 rhs=spk1buf[:, (t_global + k0 + 1) * BL:
                                             (t_global + k0 + 1 + kn) * BL],
                                 start=True, stop=True)
                for kk in range(k0, k0 + kn):
                    tt = t_global + kk
                    mprev = m2rec[:, tt * BL:(tt + 1) * BL]
                    mcur = m2rec[:, (tt + 1) * BL:(tt + 2) * BL]
                    sprev = s2rec[:, tt * BL:(tt + 1) * BL]
                    scur = s2rec[:, (tt + 1) * BL:(tt + 2) * BL]
                    nc.vector.scalar_tensor_tensor(
                        mcur, mprev, BETA2, sprev, Alu.mult, Alu.subtract)
                    nc.vector.tensor_add(
                        mcur, mcur, c2[:, (kk - k0) * BL:(kk - k0 + 1) * BL])
                    nc.vector.tensor_scalar(scur, mcur, THRESH, None, Alu.is_gt)
        t_global += nsteps

    nc.sync.dma_start(spk_o[:], s2rec[:, BL:BL + TBL])
    nc.sync.dma_start(mem_o[:], m2rec[:, BL:BL + TBL])
    stack.close()


def build_program(precision=None):
    precision = precision or PRECISION
    if precision in _PROG:
        return _PROG[precision]
    import concourse.tile as tile
    from concourse import bacc, mybir

    f32 = mybir.dt.float32
    mm_dt = {"fp32": f32, "bf16x2": mybir.dt.bfloat16}[precision]
    NS = 2 if precision == "bf16x2" else 1
    nc = bacc.Bacc("TRN2", target_bir_lowering=False, debug=False,
                   num_devices=NCORES)
    xts = [nc.dram_tensor(f"xt{j}", [KP, NS * KCH * cols], mm_dt,
                          kind="ExternalInput").ap()
           for j, cols in enumerate(COL_TILES)]
    w1f = nc.dram_tensor("w1f", [KP, NS * KCH * NH], mm_dt,
                         kind="ExternalInput").ap()
    a1 = nc.dram_tensor("a1", [KP, NH], f32, kind="ExternalInput").ap()
    w2a = nc.dram_tensor("w2a", [KP, NOUT], f32, kind="ExternalInput").ap()
    b1 = nc.dram_tensor("b1", [NH, 1], f32, kind="ExternalInput").ap()
    bet = nc.dram_tensor("bet", [NH, 1], f32, kind="ExternalInput").ap()
    s1init = nc.dram_tensor("s1init", [KP, BL + TBL], f32,
                            kind="ExternalInput").ap()
    spk_o = nc.dram_tensor("spk", [NOUT, TBL], f32, kind="ExternalOutput").ap()
    mem_o = nc.dram_tensor("mem", [NOUT, TBL], f32, kind="ExternalOutput").ap()
    aps = (xts, w1f, a1, w2a, b1, bet, s1init, spk_o, mem_o)
    with tile.TileContext(nc) as tc:
        _build_body(tc, nc, mybir, aps, precision)
    nc.compile()
    _PROG[precision] = nc
    return nc


def _mm_layout(kxn, nsplit):
    """[K=NIN, N] fp32 -> [128, nsplit*KCH*N] in matmul-ready order
    (chunk-major, hi|lo interleaved per chunk)."""
    import ml_dtypes
    n = kxn.shape[1]
    v = np.ascontiguousarray(
        kxn.reshape(KCH, KP, n).transpose(1, 0, 2))     # [128, KCH, n]
    if nsplit == 1:
        return v.reshape(KP, KCH * n)
    hi = v.astype(ml_dtypes.bfloat16)
    lo = (v - hi.astype(np.float32)).astype(ml_dtypes.bfloat16)
    out = np.empty((KP, KCH, 2, n), hi.dtype)
    out[:, :, 0, :] = hi
    out[:, :, 1, :] = lo
    return np.ascontiguousarray(out).reshape(KP, 2 * KCH * n)


def prep_inputs(x, W1, b1, beta1, V, Vb, W2, b2, precision=None):
    """Host-side shard + layout prep. Returns list of per-core input dicts."""
    precision = precision or PRECISION
    nsplit = 2 if precision == "bf16x2" else 1
    f32 = np.float32
    w1f = _mm_layout(np.ascontiguousarray(W1.T, dtype=f32), nsplit)
    a1 = np.zeros((KP, NH), f32)
    a1[:NH] = (V - THRESH * np.eye(NH, dtype=f32)).T
    a1[NH] = Vb
    w2a = np.zeros((KP, NOUT), f32)
    w2a[:NH] = W2.T
    w2a[NH] = b2
    b1a = np.ascontiguousarray(b1.reshape(NH, 1), dtype=f32)
    beta = (-np.clip(beta1, 0.0, 1.0)).astype(f32).reshape(NH, 1)  # negated
    s1init = np.zeros((KP, BL + TBL), f32)
    s1init[NH] = 1.0
    # x: [T, B, NIN] -> per-core column tiles in matmul-ready layout
    xt_full = np.ascontiguousarray(x.transpose(2, 0, 1))        # [NIN, T, B]
    col_edges = np.cumsum([0] + COL_TILES)
    in_maps = []
    for c in range(NCORES):
        xTc = np.ascontiguousarray(
            xt_full[:, :, c * BL:(c + 1) * BL]).reshape(NIN, TBL)
        m = dict(w1f=w1f, a1=a1, w2a=w2a, b1=b1a, bet=beta, s1init=s1init)
        for j, cols in enumerate(COL_TILES):
            m[f"xt{j}"] = _mm_layout(
                np.ascontiguousarray(xTc[:, col_edges[j]:col_edges[j + 1]]),
                nsplit)
        in_maps.append(m)
    return in_maps


def gather_outputs(results):
    """results: list of per-core {'spk': [NOUT, TBL], 'mem': [NOUT, TBL]}."""
    spks, mems = [], []
    for r in results:
        spks.append(np.ascontiguousarray(
            r["spk"].reshape(NOUT, T, BL).transpose(1, 2, 0)))
        mems.append(np.ascontiguousarray(
            r["mem"].reshape(NOUT, T, BL).transpose(1, 2, 0)))
    spk = np.concatenate(spks, axis=1)
    mem = np.concatenate(mems, axis=1)
    return spk.astype(np.float32), mem.astype(np.float32)


def kernel(x, W1, b1, beta1, V, Vb, W2, b2, **_run_kwargs):
    from concourse import bass_utils

    precision = _run_kwargs.pop("precision", None) or PRECISION
    nc = build_program(precision)
    in_maps = prep_inputs(np.asarray(x, np.float32), np.asarray(W1, np.float32),
                          np.asarray(b1, np.float32), np.asarray(beta1, np.float32),
                          np.asarray(V, np.float32), np.asarray(Vb, np.float32),
                          np.asarray(W2, np.float32), np.asarray(b2, np.float32),
                          precision)
    res = bass_utils.run_bass_kernel_spmd(
        nc, in_maps, core_ids=list(range(NCORES)), **_run_kwargs)
    out = gather_outputs(res.results)
    kernel.last_result = res
    return out


# revision 25
# speedup vs baseline: 1.1993x; 1.1065x over previous
"""Trainium2 Bass kernel for nn_Net_34359738709 (spiking RNN).

Model (per timestep t, reference semantics):
    cur1  = x_t @ W1.T + b1                      # [B, NH] big matmul, t-independent
    mem1  = beta1c*mem1 + cur1 + spk1 @ V.T + Vb - spk1*THRESH
    spk1  = (mem1 - THRESH > 0)
    cur2  = spk1 @ W2.T + b2
    mem2  = BETA2*mem2 + cur2 - spk2_prev*THRESH
    spk2  = (mem2 - THRESH > 0)
outputs: (spk2_rec, mem2_rec), each [T, B, NOUT]

Strategy: data-parallel over batch (B=64 -> 8 cores x 8). The x @ W1.T
matmul (21 GFLOP) is hoisted out of the time scan and computed as
cur1.T[NH, T*BL] = W1 @ x.T, accumulated over 256 K-chunks of 128 in one
PSUM bank per column tile. fp32 accuracy at bf16 speed via a hi/lo split:
x = x_hi + x_lo, W1 = w_hi + w_lo (each bf16), cur1 ~= w_hi@x_hi +
w_hi@x_lo + w_lo@x_hi (verified exact spike pattern vs fp32). Two column
tiles (34/16 timesteps): the first tile's sequential scan overlaps the
second tile's matmuls. Per scan step, layer 1 runs one augmented
128-contraction matmul (lhsT rows 0..99 = (V-I).T, row 100 = Vb, rhs =
[spk1; 1; 0]) plus three vector ops; layer 2 uses a per-burst batched
W2 matmul then a vector-only 3-op chain per step. All inputs are
host-pre-arranged into exact SBUF layouts so DMA runs long-contiguous.
"""

import sys

if "/opt/trn_rl_repo" not in sys.path:
    sys.path.insert(0, "/opt/trn_rl_repo")

import numpy as np

# Problem shapes (hardcoded per contract)
T, B, NIN, NH, NOUT = 50, 64, 32768, 100, 11
NCORES = 8
BL = B // NCORES          # 8 batch rows per core
TBL = T * BL              # 400 columns (t-major: col = t*BL + b)
KP = 128                  # contraction partition size
KCH = NIN // KP           # 256 K-chunks
COL_TILES = [272, 128]    # ncols per column tile, each % BL == 0
X_GROUPS = [2, 2, 4] + [8] * 31   # K-chunks per x dma_start (sums to 256)
THRESH = 1.0
BETA2 = 0.9753

PRECISION = "bf16x2"      # "fp32" | "bf16x2"

_PROG = {}


def _build_body(tc, nc, mybir, aps, precision):
    f32 = mybir.dt.float32
    Alu = mybir.AluOpType
    mm_dt = {"fp32": f32, "bf16x2": mybir.dt.bfloat16}[precision]
    NS = 2 if precision == "bf16x2" else 1
    xts, w1f, a1, w2a, b1, bet, s1init, spk_o, mem_o = aps

    from contextlib import ExitStack

    stack = ExitStack()
    const_pool = stack.enter_context(tc.tile_pool(name="const", bufs=1))
    state_pool = stack.enter_context(tc.tile_pool(name="state", bufs=1))
    xpool = stack.enter_context(tc.tile_pool(name="xg", bufs=6))
    curpool = stack.enter_context(tc.tile_pool(name="cur", bufs=2))
    c2pool = stack.enter_context(tc.tile_pool(name="c2s", bufs=2))
    ps_a = stack.enter_context(tc.tile_pool(name="psa", bufs=2, space="PSUM"))
    ps_s1 = stack.enter_context(tc.tile_pool(name="pss1", bufs=2, space="PSUM"))
    ps_c2 = stack.enter_context(tc.tile_pool(name="psc2", bufs=2, space="PSUM"))

    MAXC = max(COL_TILES)
    MAXG = max(X_GROUPS)

    # ---- small constants (issued on the scalar ring AFTER W1 streaming
    # starts; only needed by the scan, ~100us into the kernel) ----
    a1sb = const_pool.tile([KP, NH], f32)
    w2sb = const_pool.tile([KP, NOUT], f32)
    b1sb = const_pool.tile([NH, 1], f32)
    betnsb = const_pool.tile([NH, 1], f32)   # NEGATED clipped beta1
    # spk1 ring buffer: col block t+1 = spk1 after step t; rows 100..127
    # carry the [1; 0-pad] augmentation for every column (from s1init).
    spk1buf = state_pool.tile([KP, BL + TBL], f32)

    def load_consts():
        nc.scalar.dma_start(a1sb[:], a1)
        nc.scalar.dma_start(w2sb[:], w2a)
        nc.scalar.dma_start(b1sb[:], b1)
        nc.scalar.dma_start(betnsb[:], bet)
        nc.scalar.dma_start(spk1buf[:], s1init)

    # W1 resident in SBUF, exact matmul layout [128, NS*KCH*NH] (bf16 hi|lo)
    w1sb = const_pool.tile([KP, NS * KCH * NH], mm_dt)

    def w1_chunk(c, s=0):
        base = (c * NS + s) * NH
        return w1sb[:, base:base + NH]

    # ---- state ----
    mem1 = state_pool.tile([NH, BL], f32)
    nc.vector.memset(mem1[:], 0.0)
    m2rec = state_pool.tile([NOUT, BL + TBL], f32)
    s2rec = state_pool.tile([NOUT, BL + TBL], f32)
    nc.vector.memset(m2rec[:, 0:BL], 0.0)
    nc.vector.memset(s2rec[:, 0:BL], 0.0)

    tmpneg = state_pool.tile([NH, BL], f32)

    t_global = 0
    for j, cols in enumerate(COL_TILES):
        xt = xts[j]           # [128, NS*KCH*cols] dram, matmul-ready
        psa = ps_a.tile([NH, MAXC], f32)
        c0 = 0
        for g, gch in enumerate(X_GROUPS):
            if j == 0:
                # stream the matching W1 chunk range on the scalar ring
                w0, w1n = c0 * NS * NH, (c0 + gch) * NS * NH
                nc.scalar.dma_start(w1sb[:, w0:w1n], w1f[:, w0:w1n])
                if g == 4:
                    load_consts()
            xg = xpool.tile([KP, NS * MAXG * MAXC], mm_dt)
            gsz = NS * gch * cols
            dma_eng = nc.sync if j == 0 else (nc.sync, nc.scalar)[g % 2]
            dma_eng.dma_start(xg[:, :gsz], xt[:, c0 * NS * cols:(c0 + gch) * NS * cols])

            if NS == 1:
                for ci in range(gch):
                    c = c0 + ci
                    nc.tensor.matmul(
                        psa[:, :cols], lhsT=w1_chunk(c),
                        rhs=xg[:, ci * cols:(ci + 1) * cols],
                        start=(c == 0), stop=(c == KCH - 1))
            else:
                # hi/lo split: hh + hl + lh into one PSUM bank
                for ci in range(gch):
                    c = c0 + ci
                    nc.tensor.matmul(
                        psa[:, :cols], lhsT=w1_chunk(c, 0),
                        rhs=xg[:, ci * 2 * cols:ci * 2 * cols + cols],
                        start=(c == 0), stop=False)
                    nc.tensor.matmul(
                        psa[:, :cols], lhsT=w1_chunk(c, 0),
                        rhs=xg[:, ci * 2 * cols + cols:(ci + 1) * 2 * cols],
                        start=False, stop=False)
                    nc.tensor.matmul(
                        psa[:, :cols], lhsT=w1_chunk(c, 1),
                        rhs=xg[:, ci * 2 * cols:ci * 2 * cols + cols],
                        start=False, stop=(c == KCH - 1))
            c0 += gch
        cur = curpool.tile([NH, MAXC], f32)
        nc.vector.tensor_scalar_add(cur[:, :cols], psa[:, :cols], b1sb[:, 0:1])

        # ---- scan for this tile's timesteps ----
        # Layer 1 (PE+DVE critical loop):
        #   tmpneg = -beta*mem1 - cur_t   (independent of the V matmul)
        #   spk1   = (rec - 1) > tmpneg   (single fused op after the matmul)
        #   mem1   = rec - tmpneg
        # Layer 2 (off the critical path): per 8 steps one batched W2 matmul
        # (PE, interleaved), PSUM->SBUF copy on ScalarE, then a 3-op chain
        # per step on GpSimd.
        nsteps = cols // BL
        nc.vector.scalar_tensor_tensor(
            tmpneg[:], mem1[:], betnsb[:, 0:1], cur[:, 0:BL],
            Alu.mult, Alu.subtract)
        for k in range(nsteps):
            t = t_global + k
            rec = ps_s1.tile([NH, BL], f32)
            nc.tensor.matmul(rec[:], lhsT=a1sb[:, :],
                             rhs=spk1buf[:, t * BL:(t + 1) * BL],
                             start=True, stop=True)
            nc.vector.scalar_tensor_tensor(
                spk1buf[0:NH, (t + 1) * BL:(t + 2) * BL], rec[:],
                THRESH, tmpneg[:], Alu.subtract, Alu.is_gt)
            nc.vector.tensor_sub(mem1[:], rec[:], tmpneg[:])
            if k + 1 < nsteps:
                nc.vector.scalar_tensor_tensor(
                    tmpneg[:], mem1[:], betnsb[:, 0:1],
                    cur[:, (k + 1) * BL:(k + 2) * BL], Alu.mult, Alu.subtract)
            if k + 1 == nsteps or (k + 1) % 8 == 0:
                k0 = (k // 8) * 8
                kn = k + 1 - k0
                c2 = ps_c2.tile([NOUT, 8 * BL], f32)
                nc.tensor.matmul(c2[:, :kn * BL], lhsT=w2sb[:, :],
                                 rhs=spk1buf[:, (t_global + k0 + 1) * BL:
                                             (t_global + k0 + 1 + kn) * BL],
                                 start=True, stop=True)
                for kk in range(k0, k0 + kn):
                    tt = t_global + kk
                    mprev = m2rec[:, tt * BL:(tt + 1) * BL]
                    mcur = m2rec[:, (tt + 1) * BL:(tt + 2) * BL]
                    sprev = s2rec[:, tt * BL:(tt + 1) * BL]
                    scur = s2rec[:, (tt + 1) * BL:(tt + 2) * BL]
                    nc.vector.scalar_tensor_tensor(
                        mcur, mprev, BETA2, sprev, Alu.mult, Alu.subtract)
                    nc.vector.tensor_add(
                        mcur, mcur, c2[:, (kk - k0) * BL:(kk - k0 + 1) * BL])
                    nc.vector.tensor_scalar(scur, mcur, THRESH, None, Alu.is_gt)
        t_global += nsteps

    nc.sync.dma_start(spk_o[:], s2rec[:, BL:BL + TBL])
    nc.sync.dma_start(mem_o[:], m2rec[:, BL:BL + TBL])
    stack.close()


def build_program(precision=None):
    precision = precision or PRECISION
    if precision in _PROG:
        return _PROG[precision]
    import concourse.tile as tile
    from concourse import bacc, mybir

    f32 = mybir.dt.float32
    mm_dt = {"fp32": f32, "bf16x2": mybir.dt.bfloat16}[precision]
    NS = 2 if precision == "bf16x2" else 1
    nc = bacc.Bacc("TRN2", target_bir_lowering=False, debug=False,
                   num_devices=NCORES)
    xts = [nc.dram_tensor(f"xt{j}", [KP, NS * KCH * cols], mm_dt,
                          kind="ExternalInput").ap()
           for j, cols in enumerate(COL_TILES)]
    w1f = nc.dram_tensor("w1f", [KP, NS * KCH * NH], mm_dt,
                         kind="ExternalInput").ap()
    a1 = nc.dram_tensor("a1", [KP, NH], f32, kind="ExternalInput").ap()
    w2a = nc.dram_tensor("w2a", [KP, NOUT], f32, kind="ExternalInput").ap()
    b1 = nc.dram_tensor("b1", [NH, 1], f32, kind="ExternalInput").ap()
    bet = nc.dram_tensor("bet", [NH, 1], f32, kind="ExternalInput").ap()
    s1init = nc.dram_tensor("s1init", [KP, BL + TBL], f32,
                            kind="ExternalInput").ap()
    spk_o = nc.dram_tensor("spk", [NOUT, TBL], f32, kind="ExternalOutput").ap()
    mem_o = nc.dram_tensor("mem", [NOUT, TBL], f32, kind="ExternalOutput").ap()
    aps = (xts, w1f, a1, w2a, b1, bet, s1init, spk_o, mem_o)
    with tile.TileContext(nc) as tc:
        _build_body(tc, nc, mybir, aps, precision)
    nc.compile()
    _PROG[precision] = nc
    return nc


def _mm_layout(kxn, nsplit):
    """[K=NIN, N] fp32 -> [128, nsplit*KCH*N] in matmul-ready order
    (chunk-major, hi|lo interleaved per chunk)."""
    import ml_dtypes
    n = kxn.shape[1]
    v = np.ascontiguousarray(
        kxn.reshape(KCH, KP, n).transpose(1, 0, 2))     # [128, KCH, n]
    if nsplit == 1:
        return v.reshape(KP, KCH * n)
    hi = v.astype(ml_dtypes.bfloat16)
    lo = (v - hi.astype(np.float32)).astype(ml_dtypes.bfloat16)
    out = np.empty((KP, KCH, 2, n), hi.dtype)
    out[:, :, 0, :] = hi
    out[:, :, 1, :] = lo
    return np.ascontiguousarray(out).reshape(KP, 2 * KCH * n)


def prep_inputs(x, W1, b1, beta1, V, Vb, W2, b2, precision=None):
    """Host-side shard + layout prep. Returns list of per-core input dicts."""
    precision = precision or PRECISION
    nsplit = 2 if precision == "bf16x2" else 1
    f32 = np.float32
    w1f = _mm_layout(np.ascontiguousarray(W1.T, dtype=f32), nsplit)
    a1 = np.zeros((KP, NH), f32)
    a1[:NH] = (V - THRESH * np.eye(NH, dtype=f32)).T
    a1[NH] = Vb
    w2a = np.zeros((KP, NOUT), f32)
    w2a[:NH] = W2.T
    w2a[NH] = b2
    b1a = np.ascontiguousarray(b1.reshape(NH, 1), dtype=f32)
    beta = (-np.clip(beta1, 0.0, 1.0)).astype(f32).reshape(NH, 1)  # negated
    s1init = np.zeros((KP, BL + TBL), f32)
    s1init[NH] = 1.0
    # x: [T, B, NIN] -> per-core column tiles in matmul-ready layout
    xt_full = np.ascontiguousarray(x.transpose(2, 0, 1))        # [NIN, T, B]
    col_edges = np.cumsum([0] + COL_TILES)
    in_maps = []
    for c in range(NCORES):
        xTc = np.ascontiguousarray(
            xt_full[:, :, c * BL:(c + 1) * BL]).reshape(NIN, TBL)
        m = dict(w1f=w1f, a1=a1, w2a=w2a, b1=b1a, bet=beta, s1init=s1init)
        for j, cols in enumerate(COL_TILES):
            m[f"xt{j}"] = _mm_layout(
                np.ascontiguousarray(xTc[:, col_edges[j]:col_edges[j + 1]]),
                nsplit)
        in_maps.append(m)
    return in_maps


def gather_outputs(results):
    """results: list of per-core {'spk': [NOUT, TBL], 'mem': [NOUT, TBL]}."""
    spks, mems = [], []
    for r in results:
        spks.append(np.ascontiguousarray(
            r["spk"].reshape(NOUT, T, BL).transpose(1, 2, 0)))
        mems.append(np.ascontiguousarray(
            r["mem"].reshape(NOUT, T, BL).transpose(1, 2, 0)))
    spk = np.concatenate(spks, axis=1)
    mem = np.concatenate(mems, axis=1)
    return spk.astype(np.float32), mem.astype(np.float32)


def kernel(x, W1, b1, beta1, V, Vb, W2, b2, **_run_kwargs):
    from concourse import bass_utils

    precision = _run_kwargs.pop("precision", None) or PRECISION
    nc = build_program(precision)
    in_maps = prep_inputs(np.asarray(x, np.float32), np.asarray(W1, np.float32),
                          np.asarray(b1, np.float32), np.asarray(beta1, np.float32),
                          np.asarray(V, np.float32), np.asarray(Vb, np.float32),
                          np.asarray(W2, np.float32), np.asarray(b2, np.float32),
                          precision)
    res = bass_utils.run_bass_kernel_spmd(
        nc, in_maps, core_ids=list(range(NCORES)), **_run_kwargs)
    out = gather_outputs(res.results)
    kernel.last_result = res
    return out


# revision 26
# speedup vs baseline: 1.2105x; 1.0093x over previous
"""Trainium2 Bass kernel for nn_Net_34359738709 (spiking RNN).

Model (per timestep t, reference semantics):
    cur1  = x_t @ W1.T + b1                      # [B, NH] big matmul, t-independent
    mem1  = beta1c*mem1 + cur1 + spk1 @ V.T + Vb - spk1*THRESH
    spk1  = (mem1 - THRESH > 0)
    cur2  = spk1 @ W2.T + b2
    mem2  = BETA2*mem2 + cur2 - spk2_prev*THRESH
    spk2  = (mem2 - THRESH > 0)
outputs: (spk2_rec, mem2_rec), each [T, B, NOUT]

Strategy: data-parallel over batch (B=64 -> 8 cores x 8). The x @ W1.T
matmul (21 GFLOP) is hoisted out of the time scan and computed as
cur1.T[NH, T*BL] = W1 @ x.T, accumulated over 256 K-chunks of 128 in one
PSUM bank per column tile. fp32 accuracy at bf16 speed via a hi/lo split:
x = x_hi + x_lo, W1 = w_hi + w_lo (each bf16), cur1 ~= w_hi@x_hi +
w_hi@x_lo + w_lo@x_hi (verified exact spike pattern vs fp32). Two column
tiles (34/16 timesteps): the first tile's sequential scan overlaps the
second tile's matmuls. Per scan step, layer 1 runs one augmented
128-contraction matmul (lhsT rows 0..99 = (V-I).T, row 100 = Vb, rhs =
[spk1; 1; 0]) plus three vector ops; layer 2 uses a per-burst batched
W2 matmul then a vector-only 3-op chain per step. All inputs are
host-pre-arranged into exact SBUF layouts so DMA runs long-contiguous.
"""

import sys

if "/opt/trn_rl_repo" not in sys.path:
    sys.path.insert(0, "/opt/trn_rl_repo")

import numpy as np

# Problem shapes (hardcoded per contract)
T, B, NIN, NH, NOUT = 50, 64, 32768, 100, 11
NCORES = 8
BL = B // NCORES          # 8 batch rows per core
TBL = T * BL              # 400 columns (t-major: col = t*BL + b)
KP = 128                  # contraction partition size
KCH = NIN // KP           # 256 K-chunks
COL_TILES = [256, 144]    # ncols per column tile, each % BL == 0
X_GROUPS = [2, 2, 4] + [8] * 31   # K-chunks per x dma_start (sums to 256)
THRESH = 1.0
BETA2 = 0.9753

PRECISION = "bf16x2"      # "fp32" | "bf16x2"

_PROG = {}


def _build_body(tc, nc, mybir, aps, precision):
    f32 = mybir.dt.float32
    Alu = mybir.AluOpType
    mm_dt = {"fp32": f32, "bf16x2": mybir.dt.bfloat16}[precision]
    NS = 2 if precision == "bf16x2" else 1
    xts, w1f, a1, w2a, b1, bet, s1init, spk_o, mem_o = aps

    from contextlib import ExitStack

    stack = ExitStack()
    const_pool = stack.enter_context(tc.tile_pool(name="const", bufs=1))
    state_pool = stack.enter_context(tc.tile_pool(name="state", bufs=1))
    xpool = stack.enter_context(tc.tile_pool(name="xg", bufs=6))
    curpool = stack.enter_context(tc.tile_pool(name="cur", bufs=2))
    c2pool = stack.enter_context(tc.tile_pool(name="c2s", bufs=2))
    ps_a = stack.enter_context(tc.tile_pool(name="psa", bufs=2, space="PSUM"))
    ps_b = stack.enter_context(tc.tile_pool(name="psb", bufs=2, space="PSUM"))
    ps_s1 = stack.enter_context(tc.tile_pool(name="pss1", bufs=2, space="PSUM"))
    ps_c2 = stack.enter_context(tc.tile_pool(name="psc2", bufs=2, space="PSUM"))

    MAXC = max(COL_TILES)
    MAXG = max(X_GROUPS)

    # ---- small constants (issued on the scalar ring AFTER W1 streaming
    # starts; only needed by the scan, ~100us into the kernel) ----
    a1sb = const_pool.tile([KP, NH], f32)
    w2sb = const_pool.tile([KP, NOUT], f32)
    b1sb = const_pool.tile([NH, 1], f32)
    betnsb = const_pool.tile([NH, 1], f32)   # NEGATED clipped beta1
    # spk1 ring buffer: col block t+1 = spk1 after step t; rows 100..127
    # carry the [1; 0-pad] augmentation for every column (from s1init).
    spk1buf = state_pool.tile([KP, BL + TBL], f32)

    def load_consts():
        nc.scalar.dma_start(a1sb[:], a1)
        nc.scalar.dma_start(w2sb[:], w2a)
        nc.scalar.dma_start(b1sb[:], b1)
        nc.scalar.dma_start(betnsb[:], bet)
        nc.scalar.dma_start(spk1buf[:], s1init)

    # W1 resident in SBUF, exact matmul layout [128, NS*KCH*NH] (bf16 hi|lo)
    w1sb = const_pool.tile([KP, NS * KCH * NH], mm_dt)

    def w1_chunk(c, s=0):
        base = (c * NS + s) * NH
        return w1sb[:, base:base + NH]

    # ---- state ----
    mem1 = state_pool.tile([NH, BL], f32)
    nc.vector.memset(mem1[:], 0.0)
    m2rec = state_pool.tile([NOUT, BL + TBL], f32)
    s2rec = state_pool.tile([NOUT, BL + TBL], f32)
    nc.vector.memset(m2rec[:, 0:BL], 0.0)
    nc.vector.memset(s2rec[:, 0:BL], 0.0)

    tmpneg = state_pool.tile([NH, BL], f32)

    t_global = 0
    for j, cols in enumerate(COL_TILES):
        xt = xts[j]           # [128, NS*KCH*cols] dram, matmul-ready
        # psa accumulates [w_hi@x_hi | w_hi@x_lo] (N=2*cols); psb w_lo@x_hi
        psa = ps_a.tile([NH, 2 * MAXC], f32)
        psb = ps_b.tile([NH, MAXC], f32)
        c0 = 0
        for g, gch in enumerate(X_GROUPS):
            if j == 0:
                # stream the matching W1 chunk range on the scalar ring
                w0, w1n = c0 * NS * NH, (c0 + gch) * NS * NH
                nc.scalar.dma_start(w1sb[:, w0:w1n], w1f[:, w0:w1n])
                if g == 4:
                    load_consts()
            xg = xpool.tile([KP, NS * MAXG * MAXC], mm_dt)
            gsz = NS * gch * cols
            dma_eng = nc.sync if j == 0 else (nc.sync, nc.scalar)[g % 2]
            dma_eng.dma_start(xg[:, :gsz], xt[:, c0 * NS * cols:(c0 + gch) * NS * cols])

            if NS == 1:
                for ci in range(gch):
                    c = c0 + ci
                    nc.tensor.matmul(
                        psa[:, :cols], lhsT=w1_chunk(c),
                        rhs=xg[:, ci * cols:(ci + 1) * cols],
                        start=(c == 0), stop=(c == KCH - 1))
            else:
                # hi/lo split: one MM covers hh|hl (concat cols), one lh.
                # All w_hi MMs of the group first, then all w_lo MMs, so the
                # PSUM write bank switches once per group, not per chunk.
                for ci in range(gch):
                    c = c0 + ci
                    nc.tensor.matmul(
                        psa[:, :2 * cols], lhsT=w1_chunk(c, 0),
                        rhs=xg[:, ci * 2 * cols:(ci + 1) * 2 * cols],
                        start=(c == 0), stop=(c == KCH - 1))
                for ci in range(gch):
                    c = c0 + ci
                    nc.tensor.matmul(
                        psb[:, :cols], lhsT=w1_chunk(c, 1),
                        rhs=xg[:, ci * 2 * cols:ci * 2 * cols + cols],
                        start=(c == 0), stop=(c == KCH - 1))
            c0 += gch
        cur = curpool.tile([NH, MAXC], f32)
        nc.vector.tensor_scalar_add(cur[:, :cols], psa[:, :cols], b1sb[:, 0:1])
        if NS == 2:
            nc.vector.tensor_add(cur[:, :cols], cur[:, :cols],
                                 psa[:, cols:2 * cols])
            nc.vector.tensor_add(cur[:, :cols], cur[:, :cols], psb[:, :cols])

        # ---- scan for this tile's timesteps ----
        # Layer 1 (PE+DVE critical loop):
        #   tmpneg = -beta*mem1 - cur_t   (independent of the V matmul)
        #   spk1   = (rec - 1) > tmpneg   (single fused op after the matmul)
        #   mem1   = rec - tmpneg
        # Layer 2 (off the critical path): per 8 steps one batched W2 matmul
        # (PE, interleaved), PSUM->SBUF copy on ScalarE, then a 3-op chain
        # per step on GpSimd.
        nsteps = cols // BL
        nc.vector.scalar_tensor_tensor(
            tmpneg[:], mem1[:], betnsb[:, 0:1], cur[:, 0:BL],
            Alu.mult, Alu.subtract)
        for k in range(nsteps):
            t = t_global + k
            rec = ps_s1.tile([NH, BL], f32)
            nc.tensor.matmul(rec[:], lhsT=a1sb[:, :],
                             rhs=spk1buf[:, t * BL:(t + 1) * BL],
                             start=True, stop=True)
            nc.vector.scalar_tensor_tensor(
                spk1buf[0:NH, (t + 1) * BL:(t + 2) * BL], rec[:],
                THRESH, tmpneg[:], Alu.subtract, Alu.is_gt)
            nc.vector.tensor_sub(mem1[:], rec[:], tmpneg[:])
            if k + 1 < nsteps:
                nc.vector.scalar_tensor_tensor(
                    tmpneg[:], mem1[:], betnsb[:, 0:1],
                    cur[:, (k + 1) * BL:(k + 2) * BL], Alu.mult, Alu.subtract)
            if k + 1 == nsteps or (k + 1) % 8 == 0:
                k0 = (k // 8) * 8
                kn = k + 1 - k0
                c2 = ps_c2.tile([NOUT, 8 * BL], f32)
                nc.tensor.matmul(c2[:, :kn * BL], lhsT=w2sb[:, :],
                                 rhs=spk1buf[:, (t_global + k0 + 1) * BL:
                                             (t_global + k0 + 1 + kn) * BL],
                                 start=True, stop=True)
                for kk in range(k0, k0 + kn):
                    tt = t_global + kk
                    mprev = m2rec[:, tt * BL:(tt + 1) * BL]
                    mcur = m2rec[:, (tt + 1) * BL:(tt + 2) * BL]
                    sprev = s2rec[:, tt * BL:(tt + 1) * BL]
                    scur = s2rec[:, (tt + 1) * BL:(tt + 2) * BL]
                    nc.vector.scalar_tensor_tensor(
                        mcur, mprev, BETA2, sprev, Alu.mult, Alu.subtract)
                    nc.vector.tensor_add(
                        mcur, mcur, c2[:, (kk - k0) * BL:(kk - k0 + 1) * BL])
                    nc.vector.tensor_scalar(scur, mcur, THRESH, None, Alu.is_gt)
        t_global += nsteps

    nc.sync.dma_start(spk_o[:], s2rec[:, BL:BL + TBL])
    nc.sync.dma_start(mem_o[:], m2rec[:, BL:BL + TBL])
    stack.close()


def build_program(precision=None):
    precision = precision or PRECISION
    if precision in _PROG:
        return _PROG[precision]
    import concourse.tile as tile
    from concourse import bacc, mybir

    f32 = mybir.dt.float32
    mm_dt = {"fp32": f32, "bf16x2": mybir.dt.bfloat16}[precision]
    NS = 2 if precision == "bf16x2" else 1
    nc = bacc.Bacc("TRN2", target_bir_lowering=False, debug=False,
                   num_devices=NCORES)
    xts = [nc.dram_tensor(f"xt{j}", [KP, NS * KCH * cols], mm_dt,
                          kind="ExternalInput").ap()
           for j, cols in enumerate(COL_TILES)]
    w1f = nc.dram_tensor("w1f", [KP, NS * KCH * NH], mm_dt,
                         kind="ExternalInput").ap()
    a1 = nc.dram_tensor("a1", [KP, NH], f32, kind="ExternalInput").ap()
    w2a = nc.dram_tensor("w2a", [KP, NOUT], f32, kind="ExternalInput").ap()
    b1 = nc.dram_tensor("b1", [NH, 1], f32, kind="ExternalInput").ap()
    bet = nc.dram_tensor("bet", [NH, 1], f32, kind="ExternalInput").ap()
    s1init = nc.dram_tensor("s1init", [KP, BL + TBL], f32,
                            kind="ExternalInput").ap()
    spk_o = nc.dram_tensor("spk", [NOUT, TBL], f32, kind="ExternalOutput").ap()
    mem_o = nc.dram_tensor("mem", [NOUT, TBL], f32, kind="ExternalOutput").ap()
    aps = (xts, w1f, a1, w2a, b1, bet, s1init, spk_o, mem_o)
    with tile.TileContext(nc) as tc:
        _build_body(tc, nc, mybir, aps, precision)
    nc.compile()
    _PROG[precision] = nc
    return nc


def _mm_layout(kxn, nsplit):
    """[K=NIN, N] fp32 -> [128, nsplit*KCH*N] in matmul-ready order
    (chunk-major, hi|lo interleaved per chunk)."""
    import ml_dtypes
    n = kxn.shape[1]
    v = np.ascontiguousarray(
        kxn.reshape(KCH, KP, n).transpose(1, 0, 2))     # [128, KCH, n]
    if nsplit == 1:
        return v.reshape(KP, KCH * n)
    hi = v.astype(ml_dtypes.bfloat16)
    lo = (v - hi.astype(np.float32)).astype(ml_dtypes.bfloat16)
    out = np.empty((KP, KCH, 2, n), hi.dtype)
    out[:, :, 0, :] = hi
    out[:, :, 1, :] = lo
    return np.ascontiguousarray(out).reshape(KP, 2 * KCH * n)


def prep_inputs(x, W1, b1, beta1, V, Vb, W2, b2, precision=None):
    """Host-side shard + layout prep. Returns list of per-core input dicts."""
    precision = precision or PRECISION
    nsplit = 2 if precision == "bf16x2" else 1
    f32 = np.float32
    w1f = _mm_layout(np.ascontiguousarray(W1.T, dtype=f32), nsplit)
    a1 = np.zeros((KP, NH), f32)
    a1[:NH] = (V - THRESH * np.eye(NH, dtype=f32)).T
    a1[NH] = Vb
    w2a = np.zeros((KP, NOUT), f32)
    w2a[:NH] = W2.T
    w2a[NH] = b2
    b1a = np.ascontiguousarray(b1.reshape(NH, 1), dtype=f32)
    beta = (-np.clip(beta1, 0.0, 1.0)).astype(f32).reshape(NH, 1)  # negated
    s1init = np.zeros((KP, BL + TBL), f32)
    s1init[NH] = 1.0
    # x: [T, B, NIN] -> per-core column tiles in matmul-ready layout
    xt_full = np.ascontiguousarray(x.transpose(2, 0, 1))        # [NIN, T, B]
    col_edges = np.cumsum([0] + COL_TILES)
    in_maps = []
    for c in range(NCORES):
        xTc = np.ascontiguousarray(
            xt_full[:, :, c * BL:(c + 1) * BL]).reshape(NIN, TBL)
        m = dict(w1f=w1f, a1=a1, w2a=w2a, b1=b1a, bet=beta, s1init=s1init)
        for j, cols in enumerate(COL_TILES):
            m[f"xt{j}"] = _mm_layout(
                np.ascontiguousarray(xTc[:, col_edges[j]:col_edges[j + 1]]),
                nsplit)
        in_maps.append(m)
    return in_maps


def gather_outputs(results):
    """results: list of per-core {'spk': [NOUT, TBL], 'mem': [NOUT, TBL]}."""
    spks, mems = [], []
    for r in results:
        spks.append(np.ascontiguousarray(
            r["spk"].reshape(NOUT, T, BL).transpose(1, 2, 0)))
        mems.append(np.ascontiguousarray(
            r["mem"].reshape(NOUT, T, BL).transpose(1, 2, 0)))
    spk = np.concatenate(spks, axis=1)
    mem = np.concatenate(mems, axis=1)
    return spk.astype(np.float32), mem.astype(np.float32)


def kernel(x, W1, b1, beta1, V, Vb, W2, b2, **_run_kwargs):
    from concourse import bass_utils

    precision = _run_kwargs.pop("precision", None) or PRECISION
    nc = build_program(precision)
    in_maps = prep_inputs(np.asarray(x, np.float32), np.asarray(W1, np.float32),
                          np.asarray(b1, np.float32), np.asarray(beta1, np.float32),
                          np.asarray(V, np.float32), np.asarray(Vb, np.float32),
                          np.asarray(W2, np.float32), np.asarray(b2, np.float32),
                          precision)
    res = bass_utils.run_bass_kernel_spmd(
        nc, in_maps, core_ids=list(range(NCORES)), **_run_kwargs)
    out = gather_outputs(res.results)
    kernel.last_result = res
    return out
